# revision 6
# baseline (speedup 1.0000x reference)
"""BiLSTM-CRF Trainium2 kernel (Bass/Tile), data-parallel over batch on 8
NeuronCores. Self-contained: host prep + device emission + SPMD runner.

v2 pipeline per core (16 sequences, T=512):
  embedding gather (indirect DMA, fp16) -> DMA XBAR transpose -> Wx matmuls
  (fp16), software-pipelined with TWO independent staggered recurrence
  chains (fwd and bwd), each with a minimal 6-stage serial chain:
    4x Whh matmul (PSUM accum onto ident-copied wx)
    -> one SIGMOID over all 4 gates (tanh(g) = 2*sigma(2g)-1, g-weights
       pre-doubled on host)
    -> V: t = (sig_g - 0.5)*sig_i   [scalar_tensor_tensor]
    -> V: c' = t + m2               [m2 = sig_f*c' computed in the gap;
                                     state c' = c/2]
    -> S: tc = Tanh(c', scale=2)
    -> V: h = sig_o * tc -> hist (fp16)
  The bwd-direction pad masking is free: invalid (t,b) tokens are remapped
  to a synthetic embedding row e* with W_i_b @ e* = -30, so sigma(i)=0
  freezes c=h=0 through the pad prefix.
  Emission scores are interleaved into the second half of the recurrence.
  Then a blocked Viterbi forward scan + blocked backtrace (max-plus /
  one-hot map composition in 32 chunks of 16 steps) as before.
"""
import sys
import types
import numpy as np

import concourse.bass as bass
import concourse.mybir as mybir
from concourse import tile
from concourse.vector_clock import ScopedClock
import bass_rust
from contextlib import ExitStack

F16 = mybir.dt.float16
F32 = mybir.dt.float32
I32 = mybir.dt.int32
AF = mybir.ActivationFunctionType
AX = mybir.AxisListType.X
OP = mybir.AluOpType

B_FULL, T, V, D = 128, 512, 8000, 256
NB = 16          # sequences per core
NCORES = 8


# ---------------------------------------------------------------------------
# Harness workarounds: walrus in this environment accepts only ONE sync-wait
# per instruction; split extras onto NoOps (BIR json pass) and chunk the Tile
# exit drain. Also register the NTFF profile hook shim so BASS_TRACE=1 works.
# ---------------------------------------------------------------------------
import json as _json

_SW_CTR = [0]


def _split_sync_waits(bir_json: bytes) -> bytes:
    d = _json.loads(bir_json)
    changed = False
    for fn in d.get("functions", []):
        for blk in fn.get("blocks", []):
            new_insts = []
            for inst in blk.get("instructions", []):
                si = inst.get("sync_info")
                waits = (si or {}).get("on_wait") or []
                if len(waits) > 1:
                    changed = True
                    for w in waits[:-1]:
                        _SW_CTR[0] += 1
                        nop = {
                            "engine": inst["engine"],
                            "ins": [],
                            "outs": [],
                            "name": f"I-swsplit-{_SW_CTR[0]}",
                            "opcode": "NoOp",
                            "sync_info": {"on_update": [], "on_wait": [w]},
                        }
                        if "debug" in inst:
                            nop["debug"] = inst["debug"]
                        new_insts.append(nop)
                    si["on_wait"] = [waits[-1]]
                new_insts.append(inst)
            blk["instructions"] = new_insts
    return _json.dumps(d).encode() if changed else bir_json


def _patched_drain_and_barrier(self, tick_clock, wait_clock):
    drain_inst = self.nc.sync.drain()
    wait_clock.add_sem_waits(
        drain_inst.ins, ScopedClock({None: tick_clock.global_clock})
    )
    si = drain_inst.ins.sync_info
    if si is not None and si.on_wait is not None and len(si.on_wait) > 1:
        waits = list(si.on_wait)
        drain_inst.ins.sync_info = bass_rust.SyncInfo(
            on_wait=waits[:1], on_update=list(si.on_update or [])
        )
        for i in range(1, len(waits)):
            nop = self.nc.sync.nop()
            nop.ins.sync_info = bass_rust.SyncInfo(on_wait=[waits[i]], on_update=[])
    self.nc.all_engine_barrier()
    assert self.sems is not None
    popped = self.nc._tile_sem_poison_stack.pop()
    assert popped is self._sem_poison
    self.nc.clear_and_free_semaphores(list(self.sems.allocated().values()))
    self.nc.all_engine_barrier()


_PATCHED = [False]


def _apply_patches():
    if _PATCHED[0]:
        return
    _PATCHED[0] = True
    tile.TileContext._drain_and_barrier = _patched_drain_and_barrier
    import concourse.bass_utils as _bu
    import concourse.bass2jax as _b2j

    _orig_compile = _bu.compile_bir_kernel

    def _wrapped(bir_json, tmpdir, neff_name="file.neff"):
        return _orig_compile(_split_sync_waits(bir_json), tmpdir, neff_name)

    _wrapped._swsplit_wrapped = True
    _bu.compile_bir_kernel = _wrapped
    _b2j.compile_bir_kernel = _wrapped

    if "antenv.axon_hooks" not in sys.modules:
        try:
            import trn_agent_boot.trn_boot as _tb
            _hook = _tb._ntff_profile_via_ctypes("/opt/axon/libaxon_pjrt.so")
        except Exception:
            _hook = None
        m = types.ModuleType("antenv.axon_hooks")
        m.get_axon_ntff_profile_hook = lambda: _hook
        m.set_axon_ntff_profile_hook = lambda h: None
        sys.modules["antenv.axon_hooks"] = m


def A(t, off, dims, p0=0):
    # t: pool tile AP [[rowsize, P], [1, rowsize]]. dims[0] is the partition
    # pair whose step is replaced by the tile's canonical per-partition row
    # size; off is the within-partition element offset.
    rs = t.ap[0][0]
    d = [list(x) for x in dims]
    d[0] = [rs, d[0][1]]
    return bass.AP(t.tensor, t.offset + p0 * rs + off, d)


def AD(handle, off, dims):
    return bass.AP(handle, off, [list(d) for d in dims])


def emit_crf(nc, tc, dr, pool):
    """dr: dict of DRAM handles. pool: sbuf tile pool to allocate from."""
    v = nc.vector

    # ---- V0: build T matrices ------------------------------------------
    scT = pool.tile([128, 256], F32)   # (chpos, s, c)
    # scores_dram is tok-major [8192, 4]: addr = (t*16+b)*4 + c
    for g in range(8):
        nc.sync.dma_start(
            A(scT, 0, [[1, 16], [4, 64], [1, 4]], p0=g * 16),
            AD(dr["scores"], g * 4096, [[4, 16], [64, 64], [1, 4]]),
        )
    transb_sb = pool.tile([128, 16], F32)
    nc.sync.dma_start(transb_sb[:], dr["transb16"][None, :].to_broadcast((128, 16)))
    imp_sb = pool.tile([128, 16], F32)
    nc.sync.dma_start(imp_sb[:], dr["impflat"][None, :].to_broadcast((128, 16)))
    vmask_sb = pool.tile([128, 1024], I32)
    nc.sync.dma_start(vmask_sb[:], dr["vmask128"][:])
    fromBp_sb = pool.tile([128, 4], F32)
    nc.sync.dma_start(fromBp_sb[:], dr["fromBp4"][None, :].to_broadcast((128, 4)))

    Traw = pool.tile([128, 1024], F32)  # (chpos, s, p, c)
    v.tensor_add(
        out=A(Traw, 0, [[1, 128], [256, 4], [16, 16], [4, 4], [1, 4]]),
        in0=A(scT, 0, [[1, 128], [64, 4], [4, 16], [0, 4], [1, 4]]),
        in1=A(transb_sb, 0, [[1, 128], [0, 4], [0, 16], [4, 4], [1, 4]]),
    )
    T128 = pool.tile([128, 1024], F32)
    v.select(
        out=A(T128, 0, [[1, 128], [256, 4], [16, 16], [4, 4], [1, 4]]),
        mask=A(vmask_sb, 0, [[1, 128], [256, 4], [16, 16], [4, 4], [1, 4]]),
        on_true=A(Traw, 0, [[1, 128], [256, 4], [16, 16], [4, 4], [1, 4]]),
        on_false=A(imp_sb, 0, [[1, 128], [0, 4], [0, 16], [4, 4], [1, 4]]),
    )
    # step 0 (partitions 0:16, chpos=0, s=0): T = e0 + fromBp (rows equal)
    v.tensor_add(
        out=A(T128, 0, [[1, 16], [4, 4], [1, 4]]),
        in0=A(scT, 0, [[1, 16], [0, 4], [1, 4]]),
        in1=A(fromBp_sb, 0, [[1, 16], [0, 4], [1, 4]]),
    )

    # ---- V1: chunk max-plus products -----------------------------------
    Ma = pool.tile([128, 64], F32)   # (chpos, i, k/j)
    Mb = pool.tile([128, 64], F32)
    tmp256 = pool.tile([128, 1024], F32)
    v.tensor_copy(
        A(Ma, 0, [[1, 128], [16, 4], [4, 4], [1, 4]]),
        A(T128, 0, [[1, 128], [256, 4], [4, 4], [1, 4]]),
    )
    cur, nxt = Ma, Mb
    for s in range(1, 16):
        v.tensor_add(
            out=A(tmp256, 0, [[1, 128], [64, 4], [16, 4], [4, 4], [1, 4]]),
            in0=A(cur, 0, [[1, 128], [16, 4], [4, 4], [1, 4], [0, 4]]),
            in1=A(T128, s * 16, [[1, 128], [256, 4], [0, 4], [4, 4], [1, 4]]),
        )
        v.tensor_reduce(
            out=A(nxt, 0, [[1, 128], [16, 4], [4, 4], [1, 4]]),
            in_=A(tmp256, 0, [[1, 128], [64, 4], [16, 4], [1, 4], [4, 4]]),
            axis=AX, op=OP.max,
        )
        cur, nxt = nxt, cur
    nc.sync.dma_start(
        AD(dr["mdram"], 0, [[64, 128], [1, 64]]),
        A(cur, 0, [[1, 128], [1, 64]]),
    )

    # ---- V2: serial chunk scan (16 partitions) -------------------------
    M16 = pool.tile([16, 512], F32)
    for g in range(8):
        nc.sync.dma_start(
            A(M16, g * 64, [[1, 16], [1, 64]]),
            AD(dr["mdram"], g * 1024, [[64, 16], [1, 64]]),
        )
    Ball = pool.tile([16, 132], F32)
    v.memset(Ball[:], 0.0)
    t16 = pool.tile([16, 16], F32)
    for c in range(32):
        v.tensor_add(
            out=A(t16, 0, [[1, 16], [4, 4], [1, 4]]),
            in0=A(Ball, c * 4, [[1, 16], [1, 4], [0, 4]]),
            in1=A(M16, c * 16, [[1, 16], [4, 4], [1, 4]]),
        )
        v.tensor_reduce(
            out=A(Ball, (c + 1) * 4, [[1, 16], [1, 4]]),
            in_=A(t16, 0, [[1, 16], [1, 4], [4, 4]]),
            axis=AX, op=OP.max,
        )
    # last label one-hot
    toEOS_sb = pool.tile([16, 4], F32)
    nc.sync.dma_start(toEOS_sb[:], dr["toEOS4"][None, :].to_broadcast((16, 4)))
    c3lab_sb = pool.tile([16, 4], F32)
    nc.sync.dma_start(c3lab_sb[:], dr["c3lab4"][None, :].to_broadcast((16, 4)))
    wiota16 = pool.tile([16, 4], F32)
    nc.sync.dma_start(wiota16[:], dr["wiota4"][None, :].to_broadcast((16, 4)))
    fin = pool.tile([16, 4], F32)
    v.tensor_add(out=fin[:], in0=A(Ball, 128, [[1, 16], [1, 4]]), in1=toEOS_sb[:])
    lmax = pool.tile([16, 1], F32)
    v.tensor_reduce(out=lmax[:], in_=fin[:], axis=AX, op=OP.max)
    loh = pool.tile([16, 4], F32)
    v.tensor_tensor(out=loh[:], in0=fin[:],
                    in1=A(lmax, 0, [[1, 16], [0, 4]]), op=OP.is_equal)
    lohm = pool.tile([16, 4], F32)
    v.tensor_mul(out=lohm[:], in0=loh[:], in1=c3lab_sb[:])
    lenc = pool.tile([16, 1], F32)
    v.tensor_reduce(out=lenc[:], in_=lohm[:], axis=AX, op=OP.max)
    llval = pool.tile([16, 1], F32)
    v.tensor_scalar(out=llval[:], in0=lenc[:], scalar1=-1.0, scalar2=3.0,
                    op0=OP.mult, op1=OP.add)
    lloh = pool.tile([16, 4], F32)
    v.tensor_tensor(out=lloh[:], in0=wiota16[:],
                    in1=A(llval, 0, [[1, 16], [0, 4]]), op=OP.is_equal)
    nc.sync.dma_start(AD(dr["lldram"], 0, [[4, 16], [1, 4]]), lloh[:])
    nc.sync.dma_start(AD(dr["edram"], 0, [[132, 16], [1, 132]]), Ball[:])

    # ---- V3: replay -> backtrace tables --------------------------------
    c3p_sb = pool.tile([128, 16], F32)
    nc.sync.dma_start(c3p_sb[:], dr["c3p16"][None, :].to_broadcast((128, 16)))
    bestA = pool.tile([128, 16], F32)
    bestB = pool.tile([128, 16], F32)
    nc.sync.dma_start(
        bestA[:], AD(dr["edram"], 0, [[16, 8], [132, 16], [4, 4], [1, 4]])
    )
    BT = pool.tile([128, 256], F32)     # (chpos, s, c)
    smat = pool.tile([128, 64], F32)
    oh64 = pool.tile([128, 64], F32)
    enc128 = pool.tile([128, 16], F32)
    bcur, bnxt = bestA, bestB
    for s in range(16):
        v.tensor_add(
            out=A(smat, 0, [[1, 128], [16, 4], [4, 4], [1, 4]]),
            in0=A(bcur, 0, [[1, 128], [4, 4], [1, 4], [0, 4]]),
            in1=A(T128, s * 16, [[1, 128], [256, 4], [4, 4], [1, 4]]),
        )
        v.tensor_reduce(
            out=A(bnxt, 0, [[1, 128], [4, 4], [1, 4]]),
            in_=A(smat, 0, [[1, 128], [16, 4], [1, 4], [4, 4]]),
            axis=AX, op=OP.max,
        )
        v.tensor_tensor(
            out=A(oh64, 0, [[1, 128], [16, 4], [4, 4], [1, 4]]),
            in0=A(smat, 0, [[1, 128], [16, 4], [4, 4], [1, 4]]),
            in1=A(bnxt, 0, [[1, 128], [4, 4], [0, 4], [1, 4]]),
            op=OP.is_equal,
        )
        v.tensor_mul(
            out=A(oh64, 0, [[1, 128], [16, 4], [4, 4], [1, 4]]),
            in0=A(oh64, 0, [[1, 128], [16, 4], [4, 4], [1, 4]]),
            in1=A(c3p_sb, 0, [[1, 128], [0, 4], [4, 4], [1, 4]]),
        )
        v.tensor_reduce(
            out=A(enc128, 0, [[1, 128], [4, 4], [1, 4]]),
            in_=A(oh64, 0, [[1, 128], [16, 4], [1, 4], [4, 4]]),
            axis=AX, op=OP.max,
        )
        v.tensor_scalar(
            out=A(BT, s * 4, [[1, 128], [64, 4], [1, 4]]),
            in0=A(enc128, 0, [[1, 128], [4, 4], [1, 4]]),
            scalar1=-1.0, scalar2=3.0, op0=OP.mult, op1=OP.add,
        )
        bcur, bnxt = bnxt, bcur
    nc.sync.dma_start(
        AD(dr["btdram"], 0, [[256, 128], [1, 256]]),
        A(BT, 0, [[1, 128], [1, 256]]),
    )

    # ---- VA: backtrace map tables + chunk compositions -----------------
    BTS = pool.tile([128, 256], F32)
    # top group's last slot is never used; zero-fill before partial overwrite
    v.memset(A(BTS, 252, [[1, 128], [1, 4]]), 0.0)
    nc.sync.dma_start(
        A(BTS, 0, [[1, 128], [1, 252]]),
        AD(dr["btdram"], 4, [[256, 128], [1, 252]]),
    )
    # last slot of each partition: first bt entry of the next chunk group
    nc.sync.dma_start(
        A(BTS, 252, [[1, 112], [1, 4]]),
        AD(dr["btdram"], 16 * 256, [[256, 112], [1, 4]]),
    )
    meq_sb = pool.tile([128, 64], I32)
    mlt_sb = pool.tile([128, 64], I32)
    nc.sync.dma_start(meq_sb[:], dr["meq128"][:])
    nc.sync.dma_start(mlt_sb[:], dr["mlt128"][:])
    lloh128 = pool.tile([128, 4], F32)
    nc.sync.dma_start(lloh128[:], AD(dr["lldram"], 0, [[0, 8], [4, 16], [1, 4]]))
    i4_sb = pool.tile([128, 16], F32)
    nc.sync.dma_start(i4_sb[:], dr["i4flat"][None, :].to_broadcast((128, 16)))
    wiota128 = pool.tile([128, 4], F32)
    nc.sync.dma_start(wiota128[:], dr["wiota4"][None, :].to_broadcast((128, 4)))

    Fall = pool.tile([128, 1024], F32)  # (chpos, s, u, w)
    tmpA = pool.tile([128, 64], F32)
    for s in range(16):
        # oh(u,w) = bt_{t+1}[u] == w
        v.tensor_tensor(
            out=A(tmpA, 0, [[1, 128], [16, 4], [4, 4], [1, 4]]),
            in0=A(BTS, s * 4, [[1, 128], [64, 4], [1, 4], [0, 4]]),
            in1=A(wiota128, 0, [[1, 128], [0, 4], [0, 4], [1, 4]]),
            op=OP.is_equal,
        )
        # tmp2 = meq ? lloh : I4  ; F = mlt ? oh : tmp2  (write into Fall)
        v.select(
            out=A(Fall, s * 16, [[1, 128], [256, 4], [4, 4], [1, 4]]),
            mask=A(meq_sb, s, [[1, 128], [16, 4], [0, 4], [0, 4]]),
            on_true=A(lloh128, 0, [[1, 128], [0, 4], [0, 4], [1, 4]]),
            on_false=A(i4_sb, 0, [[1, 128], [0, 4], [4, 4], [1, 4]]),
        )
        v.select(
            out=A(Fall, s * 16, [[1, 128], [256, 4], [4, 4], [1, 4]]),
            mask=A(mlt_sb, s, [[1, 128], [16, 4], [0, 4], [0, 4]]),
            on_true=A(tmpA, 0, [[1, 128], [16, 4], [4, 4], [1, 4]]),
            on_false=A(Fall, s * 16, [[1, 128], [256, 4], [4, 4], [1, 4]]),
        )
    Ga = pool.tile([128, 64], F32)
    Gb = pool.tile([128, 64], F32)
    v.tensor_copy(
        A(Ga, 0, [[1, 128], [16, 4], [4, 4], [1, 4]]),
        A(Fall, 15 * 16, [[1, 128], [256, 4], [4, 4], [1, 4]]),
    )
    gcur, gnxt = Ga, Gb
    for s in range(14, -1, -1):
        v.tensor_mul(
            out=A(tmp256, 0, [[1, 128], [64, 4], [16, 4], [4, 4], [1, 4]]),
            in0=A(gcur, 0, [[1, 128], [16, 4], [4, 4], [1, 4], [0, 4]]),
            in1=A(Fall, s * 16, [[1, 128], [256, 4], [0, 4], [4, 4], [1, 4]]),
        )
        v.tensor_reduce(
            out=A(gnxt, 0, [[1, 128], [16, 4], [4, 4], [1, 4]]),
            in_=A(tmp256, 0, [[1, 128], [64, 4], [16, 4], [1, 4], [4, 4]]),
            axis=AX, op=OP.max,
        )
        gcur, gnxt = gnxt, gcur
    nc.sync.dma_start(
        AD(dr["gdram"], 0, [[64, 128], [1, 64]]),
        A(gcur, 0, [[1, 128], [1, 64]]),
    )

    # ---- VB: serial reverse chunk scan (16 partitions) -----------------
    Gall16 = pool.tile([16, 512], F32)
    for g in range(8):
        nc.sync.dma_start(
            A(Gall16, g * 64, [[1, 16], [1, 64]]),
            AD(dr["gdram"], g * 1024, [[64, 16], [1, 64]]),
        )
    EB = pool.tile([16, 132], F32)
    nc.sync.dma_start(
        A(EB, 128, [[1, 16], [1, 4]]), dr["e0oh4"][None, :].to_broadcast((16, 4))
    )
    tb16 = pool.tile([16, 16], F32)
    for c in range(31, -1, -1):
        v.tensor_mul(
            out=tb16[:],
            in0=A(EB, (c + 1) * 4, [[1, 16], [1, 4], [0, 4]]),
            in1=A(Gall16, c * 16, [[1, 16], [4, 4], [1, 4]]),
        )
        v.tensor_reduce(
            out=A(EB, c * 4, [[1, 16], [1, 4]]),
            in_=A(tb16, 0, [[1, 16], [1, 4], [4, 4]]),
            axis=AX, op=OP.max,
        )
    nc.sync.dma_start(AD(dr["ebdram"], 0, [[132, 16], [1, 132]]), EB[:])

    # ---- VC: labels -----------------------------------------------------
    cohE = pool.tile([128, 16], F32)
    nc.sync.dma_start(
        cohE[:], AD(dr["ebdram"], 4, [[16, 8], [132, 16], [4, 4], [1, 4]])
    )
    LABOH = pool.tile([128, 256], F32)  # (chpos, s, w)
    tmpc = pool.tile([128, 64], F32)
    for s in range(15, -1, -1):
        if s == 15:
            in0 = A(cohE, 0, [[1, 128], [4, 4], [1, 4], [0, 4]])
        else:
            in0 = A(LABOH, (s + 1) * 4, [[1, 128], [64, 4], [1, 4], [0, 4]])
        v.tensor_mul(
            out=A(tmpc, 0, [[1, 128], [16, 4], [4, 4], [1, 4]]),
            in0=in0,
            in1=A(Fall, s * 16, [[1, 128], [256, 4], [4, 4], [1, 4]]),
        )
        v.tensor_reduce(
            out=A(LABOH, s * 4, [[1, 128], [64, 4], [1, 4]]),
            in_=A(tmpc, 0, [[1, 128], [16, 4], [1, 4], [4, 4]]),
            axis=AX, op=OP.max,
        )
    omask_sb = pool.tile([128, 64], F32)
    nc.sync.dma_start(omask_sb[:], dr["outmask128"][:])
    labv = pool.tile([128, 64], F32)
    tmpl = pool.tile([128, 256], F32)
    v.tensor_mul(
        out=A(tmpl, 0, [[1, 128], [64, 4], [4, 16], [1, 4]]),
        in0=A(LABOH, 0, [[1, 128], [64, 4], [4, 16], [1, 4]]),
        in1=A(wiota128, 0, [[1, 128], [0, 4], [0, 16], [1, 4]]),
    )
    v.tensor_reduce(
        out=A(labv, 0, [[1, 128], [16, 4], [1, 16]]),
        in_=A(tmpl, 0, [[1, 128], [64, 4], [4, 16], [1, 4]]),
        axis=AX, op=OP.add,
    )
    v.tensor_mul(out=labv[:], in0=labv[:], in1=omask_sb[:])
    labi = pool.tile([128, 64], I32)
    v.tensor_copy(labi[:], labv[:])
    for cp in range(4):
        nc.sync.dma_start(
            AD(dr["labels"], 16 * cp, [[64, 8], [512, 16], [1, 16]]),
            A(labi, cp * 16, [[1, 128], [1, 16]]),
        )


def host_crf_consts(lens, trans, fromB, toEOS, b_lab):
    """All host-side constant arrays, keyed to match dram handle names."""
    import numpy as np
    T, B, L = 512, 16, 4
    NEG = -1e9
    out = {}
    out["transb16"] = (trans + b_lab[None, :]).astype(np.float32).reshape(16)
    imp = np.full((L, L), NEG, np.float32)
    np.fill_diagonal(imp, 0.0)
    out["impflat"] = imp.reshape(16)
    out["fromBp4"] = (fromB + b_lab).astype(np.float32)
    out["toEOS4"] = toEOS.astype(np.float32)
    out["c3lab4"] = (3.0 - np.arange(4)).astype(np.float32)
    out["wiota4"] = np.arange(4).astype(np.float32)
    out["c3p16"] = np.repeat(3.0 - np.arange(4), 4).astype(np.float32)
    out["i4flat"] = np.eye(4, dtype=np.float32).reshape(16)
    out["e0oh4"] = np.array([1, 0, 0, 0], np.float32)
    # t value at (P, chpos, s):  P = chgrp*16 + b ; t = 16*(4*chgrp+chpos)+s
    P_chgrp = np.arange(128) // 16
    P_b = np.arange(128) % 16
    chpos = np.arange(4)
    s = np.arange(16)
    tt = 16 * (4 * P_chgrp[:, None, None] + chpos[None, :, None]) + s[None, None, :]
    lb = lens[P_b][:, None, None]
    vm = (tt < lb)
    out["vmask128"] = np.repeat(
        vm.reshape(128, 64)[:, :, None], 16, axis=2
    ).reshape(128, 1024).astype(np.int32)
    out["meq128"] = (tt == lb - 1).reshape(128, 64).astype(np.int32)
    out["mlt128"] = (tt < lb - 1).reshape(128, 64).astype(np.int32)
    out["outmask128"] = (tt < lb).reshape(128, 64).astype(np.float32)
    return out


CRF_DRAM_SPECS = [
    ("transb16", [16], F32), ("impflat", [16], F32), ("fromBp4", [4], F32),
    ("toEOS4", [4], F32), ("c3lab4", [4], F32), ("wiota4", [4], F32),
    ("c3p16", [16], F32), ("i4flat", [16], F32), ("e0oh4", [4], F32),
    ("vmask128", [128, 1024], I32), ("meq128", [128, 64], I32),
    ("mlt128", [128, 64], I32), ("outmask128", [128, 64], F32),
]
CRF_SCRATCH_SPECS = [
    ("mdram", [8192], F32), ("edram", [2112], F32), ("btdram", [32832], F32),
    ("gdram", [8192], F32), ("lldram", [64], F32), ("ebdram", [2112], F32),
]


class LstmEmitter:
    """Two staggered chains (f, b), minimal per-step serial chain."""

    def __init__(self, nc, tc, dr, T, pools):
        self.nc, self.tc, self.dr, self.T = nc, tc, dr, T
        self.NBLK = T // 64
        p = pools
        self.hist = {}
        for d in ("f", "b"):
            h = p["hist"].tile([128, (T + 1) * 16], F16, name=f"hist_{d}")
            self.hist[d] = h
        nc.vector.memset(self.hist["f"][:, 0:16], 0.0)
        nc.vector.memset(self.hist["b"][:, T * 16:(T + 1) * 16], 0.0)
        # true cell state c, fp32, kept in PSUM (fast activation input)
        self.cst = {}
        for d in ("f", "b"):
            c = p["state_ps"].tile([128, 16], F32, name=f"c_{d}")
            nc.vector.memset(c[:], 0.0)
            self.cst[d] = c
        # weights
        self.whhT = {}
        self.wihT = {}
        self.biasT = {}
        self.wlabT = {}
        for d in ("f", "b"):
            w = p["wts"].tile([128, 512], F16, name=f"whh_{d}")
            nc.sync.dma_start(w[:], dr[f"whhT_{d}"][:])
            self.whhT[d] = w
            hs = []
            for h in range(2):
                wh = p["wts"].tile([128, 512], F16, name=f"wih_{d}{h}")
                nc.sync.dma_start(wh[:], dr[f"wihT_{d}{h}"][:])
                hs.append(wh)
            self.wihT[d] = hs
            bt = p["wts"].tile([128, 4], F32, name=f"bias_{d}")
            nc.sync.dma_start(bt[:], dr[f"biasT_{d}"][:])
            self.biasT[d] = bt
            wl = p["wts"].tile([128, 4], F16, name=f"wlab_{d}")
            nc.sync.dma_start(wl[:], dr[f"wlabT_{d}"][:])
            self.wlabT[d] = wl
        self.ident = p["wts"].tile([128, 128], F16)
        nc.sync.dma_start(self.ident[:], dr["ident"][:])
        self.pools = p
        self.wx = {}   # (dir, blk) -> tile [128, 4096] fp16, (j)(tin)(b)

    # ---- production of one dir-block's wx ------------------------------
    def production_items(self, d, blk):
        """Closures emitting gather / DMA-transpose / matmul / bias work
        that materializes wx[d][blk]."""
        nc, dr, p = self.nc, self.dr, self.pools
        items = []
        state = {}

        def alloc():
            state["idx"] = p["idx"].tile([128, 8], I32, name=f"idx_{d}")
            nc.sync.dma_start(
                state["idx"][:],
                bass.AP(dr[f"tokens_{d}"], blk * 1024, [[1, 128], [128, 8]]),
            )
            state["xt"] = p["xt"].tile([128, 2048], F16, name="xt")
            state["wx"] = p[f"wx_{d}"].tile([128, 4096], F16, name=f"wx_{d}")
            self.wx[(d, blk)] = state["wx"]
            state["xg"] = []

        def gather(i):
            def go():
                t = p["xg"].tile([128, 256], F16)
                nc.gpsimd.indirect_dma_start(
                    out=t[:], out_offset=None, in_=dr["emb16"][:],
                    in_offset=bass.IndirectOffsetOnAxis(
                        ap=state["idx"][:, i:i + 1], axis=0),
                )
                state["xg"].append(t)
            return go

        def transp(i, h):
            def go():
                nc.sync.dma_start(
                    state["xt"][:, h * 1024 + i * 128: h * 1024 + (i + 1) * 128],
                    state["xg"][i][:, h * 128:(h + 1) * 128],
                    transpose=True,
                )
            return go

        def mm(j, n):
            def go():
                ps = p["wx_ps"].tile([128, 512], F32)
                state[("ps", j, n)] = ps
                for h in range(2):
                    nc.tensor.matmul(
                        out=ps[:],
                        lhsT=self.wihT[d][h][:, j * 128:(j + 1) * 128],
                        rhs=state["xt"][:, h * 1024 + n * 512: h * 1024 + (n + 1) * 512],
                        start=(h == 0), stop=(h == 1),
                    )
            return go

        def bias(j, n):
            def go():
                nc.scalar.activation(
                    out=state["wx"][:, j * 1024 + n * 512: j * 1024 + (n + 1) * 512],
                    in_=state[("ps", j, n)][:],
                    func=AF.Identity, bias=self.biasT[d][:, j:j + 1],
                )
            return go

        items.append(alloc)
        for i in range(8):
            items.append(gather(i))
            for h in range(2):
                items.append(transp(i, h))
        for j in range(4):
            for n in range(2):
                items.append(mm(j, n))
                items.append(bias(j, n))
        return items

    # ---- one chain step ------------------------------------------------
    def slot(self, d, t):
        nc, p = self.nc, self.pools
        tins = t % 64
        blk = t // 64
        ha = self.hist[d]
        if d == "f":
            hprev = ha[:, t * 16:(t + 1) * 16]
            hout = ha[:, (t + 1) * 16:(t + 2) * 16]
        else:
            hprev = ha[:, (t + 1) * 16:(t + 2) * 16]
            hout = ha[:, t * 16:(t + 1) * 16]
        wxt = self.wx[(d, blk)]
        gp = p[f"g_ps_{d}"].tile([128, 64], F32, name=f"g_ps_{d}")
        # wx -> PSUM via identity matmul (cols (j,b) for this tin)
        nc.tensor.matmul(
            out=gp[:],
            lhsT=self.ident[:],
            rhs=A(wxt, tins * 16, [[1, 128], [1024, 4], [1, 16]]),
            start=True, stop=False,
        )
        for j in range(4):
            nc.tensor.matmul(
                out=gp[:, j * 16:(j + 1) * 16],
                lhsT=self.whhT[d][:, j * 128:(j + 1) * 128],
                rhs=hprev, start=False, stop=(j == 3),
            )
        # one sigmoid over all gates (g pre-doubled: tanh(g) = 2*sig(2g)-1)
        act = p[f"act_{d}"].tile([128, 64], F16, name=f"act_{d}")
        nc.scalar.activation(out=act[:], in_=gp[:], func=AF.Sigmoid)
        c = self.cst[d]
        # all chain V-ops in scalar_tensor_tensor form (faster DVE path)
        m2 = p[f"m2_{d}"].tile([128, 16], F32, name=f"m2_{d}")
        nc.vector.scalar_tensor_tensor(
            out=m2[:], in0=act[:, 16:32], scalar=0.0, in1=c[:],
            op0=OP.bypass, op1=OP.mult,
        )
        tt = p[f"t_{d}"].tile([128, 16], F32, name=f"t_{d}")
        nc.vector.scalar_tensor_tensor(
            out=tt[:], in0=act[:, 48:64], scalar=0.5, in1=act[:, 0:16],
            op0=OP.subtract, op1=OP.mult,
        )
        # c = 2*t + m2  (true cell state; tanh(g)*sig(i) = 2*t)
        nc.vector.scalar_tensor_tensor(
            out=c[:], in0=tt[:], scalar=2.0, in1=m2[:],
            op0=OP.mult, op1=OP.add,
        )
        tc_ = p[f"tc_{d}"].tile([128, 16], F32, name=f"tc_{d}")
        nc.scalar.activation(out=tc_[:], in_=c[:], func=AF.Tanh)
        nc.vector.scalar_tensor_tensor(
            out=hout, in0=act[:, 32:48], scalar=0.0, in1=tc_[:],
            op0=OP.bypass, op1=OP.mult,
        )

    # ---- emission score chunk n (tokens n*128 .. (n+1)*128) ------------
    def score_chunk(self, n):
        nc, p = self.nc, self.pools
        ps = p["sc_ps"].tile([128, 4], F32)
        nc.tensor.matmul(out=ps[:],
                         lhsT=self.hist["f"][:, 16 + n * 128: 16 + (n + 1) * 128],
                         rhs=self.wlabT["f"][:], start=True, stop=False)
        nc.tensor.matmul(out=ps[:],
                         lhsT=self.hist["b"][:, n * 128:(n + 1) * 128],
                         rhs=self.wlabT["b"][:], start=False, stop=True)
        sb = p["sc_sb"].tile([128, 4], F32)
        nc.vector.tensor_copy(sb[:], ps[:])
        nc.sync.dma_start(
            bass.AP(self.dr["scores"], n * 512, [[4, 128], [1, 4]]), sb[:]
        )

    # ---- full pipelined emission ---------------------------------------
    def emit_recurrence(self):
        T, NBLK = self.T, self.NBLK
        for it in self.production_items("f", 0):
            it()
        for it in self.production_items("b", NBLK - 1):
            it()
        emitted = [False] * (T * 16 // 128)   # score chunks
        nsc = len(emitted)

        def ready_chunks(s):
            out = []
            for n in range(nsc):
                if not emitted[n] and max(8 * n + 7, (T - 1) - 8 * n) <= s:
                    out.append(n)
            return out

        for blk in range(NBLK):
            todo = []
            if blk + 1 < NBLK:
                fa = self.production_items("f", blk + 1)
                fb = self.production_items("b", NBLK - 2 - blk)
                # interleave the two dirs' production round-robin
                for x, y in zip(fa, fb):
                    todo.append(x)
                    todo.append(y)
            k = 0
            for tin in range(64):
                s = blk * 64 + tin
                self.slot("f", s)
                want = ((2 * tin + 1) * len(todo)) // 128
                while k < want:
                    todo[k]()
                    k += 1
                self.slot("b", (T - 1) - s)
                want = ((2 * tin + 2) * len(todo)) // 128
                while k < want:
                    todo[k]()
                    k += 1
                for n in ready_chunks(s)[:2]:
                    self.score_chunk(n)
                    emitted[n] = True
            while k < len(todo):
                todo[k]()
                k += 1
        for n in range(nsc):
            if not emitted[n]:
                self.score_chunk(n)
                emitted[n] = True


def host_lstm_shared(inp):
    """Batch-independent host arrays (weights etc)."""
    shared = {}
    perm = np.concatenate([np.arange(128), 128 + np.arange(128),
                           384 + np.arange(128), 256 + np.arange(128)])
    emb = np.asarray(inp["emb"]).astype(np.float32)
    # synthetic row 8000: W_i_b @ e* = -30 for every i-gate (freezes bwd state
    # through the pad prefix: sigma(i)=0 -> c=h=0)
    W_i_b = np.asarray(inp["W_ih_b"]).astype(np.float64)[0:128]
    e_star, *_ = np.linalg.lstsq(W_i_b, np.full(128, -30.0), rcond=None)
    emb16 = np.concatenate([emb, e_star[None, :].astype(np.float32)], axis=0)
    shared["emb16"] = emb16.astype(np.float16)
    for d, sfx in (("f", "_f"), ("b", "_b")):
        wih = np.asarray(inp[f"W_ih{sfx}"]).astype(np.float32)[perm].copy()
        whh = np.asarray(inp[f"W_hh{sfx}"]).astype(np.float32)[perm].copy()
        bias = (np.asarray(inp[f"b_ih{sfx}"]) +
                np.asarray(inp[f"b_hh{sfx}"])).astype(np.float32)[perm].copy()
        # tanh gate: pre-double (tanh(g) = 2*sigmoid(2g) - 1)
        wih[384:] *= 2.0
        whh[384:] *= 2.0
        bias[384:] *= 2.0
        shared[f"wihT_{d}0"] = np.ascontiguousarray(wih.T[:128]).astype(np.float16)
        shared[f"wihT_{d}1"] = np.ascontiguousarray(wih.T[128:]).astype(np.float16)
        shared[f"whhT_{d}"] = np.ascontiguousarray(whh.T).astype(np.float16)
        shared[f"biasT_{d}"] = np.ascontiguousarray(
            bias.reshape(4, 128).T).astype(np.float32)
        wl = np.asarray(inp["W_lab"]).astype(np.float32)
        half = wl[:, :128] if d == "f" else wl[:, 128:]
        shared[f"wlabT_{d}"] = np.ascontiguousarray(half.T).astype(np.float16)
    shared["ident"] = np.eye(128, dtype=np.float16)
    return shared


def host_tokens(pad_seq, lens, T=512):
    """Per-core token arrays: fwd natural; bwd with pad positions remapped to
    the synthetic frozen-state row (8000)."""
    tok_f = np.ascontiguousarray(pad_seq.T).reshape(-1).astype(np.int32)
    tb = pad_seq.T.copy().astype(np.int32)          # [T, NB]
    invalid = np.arange(T)[:, None] >= lens[None, :]
    tb[invalid] = 8000
    tok_b = np.ascontiguousarray(tb).reshape(-1)
    return tok_f, tok_b


def lstm_dram_specs(T=512):
    return [
        ("emb16", [8001, 256], F16),
        ("tokens_f", [T * 16], I32), ("tokens_b", [T * 16], I32),
        ("wihT_f0", [128, 512], F16), ("wihT_f1", [128, 512], F16),
        ("wihT_b0", [128, 512], F16), ("wihT_b1", [128, 512], F16),
        ("whhT_f", [128, 512], F16), ("whhT_b", [128, 512], F16),
        ("biasT_f", [128, 4], F32), ("biasT_b", [128, 4], F32),
        ("wlabT_f", [128, 4], F16), ("wlabT_b", [128, 4], F16),
        ("ident", [128, 128], F16),
    ]


def make_pools(ctx_persist, ctx_trans, tc):
    p = {}
    p["hist"] = ctx_persist.enter_context(tc.tile_pool(name="hist", bufs=1))
    p["state_ps"] = ctx_persist.enter_context(
        tc.tile_pool(name="state_ps", bufs=1, space="PSUM"))
    p["wts"] = ctx_persist.enter_context(tc.tile_pool(name="wts", bufs=1))
    p["idx"] = ctx_trans.enter_context(tc.tile_pool(name="idx", bufs=4))
    p["xg"] = ctx_trans.enter_context(tc.tile_pool(name="xg", bufs=6))
    p["xt"] = ctx_trans.enter_context(tc.tile_pool(name="xt", bufs=4))
    p["wx_f"] = ctx_trans.enter_context(tc.tile_pool(name="wx_f", bufs=2))
    p["wx_b"] = ctx_trans.enter_context(tc.tile_pool(name="wx_b", bufs=2))
    p["wx_ps"] = ctx_trans.enter_context(tc.tile_pool(name="wx_ps", bufs=1, space="PSUM"))
    p["g_ps_f"] = ctx_trans.enter_context(tc.tile_pool(name="g_ps_f", bufs=2, space="PSUM"))
    p["g_ps_b"] = ctx_trans.enter_context(tc.tile_pool(name="g_ps_b", bufs=2, space="PSUM"))
    p["sc_ps"] = ctx_trans.enter_context(tc.tile_pool(name="sc_ps", bufs=1, space="PSUM"))
    p["sc_sb"] = ctx_trans.enter_context(tc.tile_pool(name="sc_sb", bufs=4))
    for d in ("f", "b"):
        for nm in ("act", "m2", "t", "tc"):
            p[f"{nm}_{d}"] = ctx_trans.enter_context(
                tc.tile_pool(name=f"{nm}_{d}", bufs=2))
    return p


# ---------------------------------------------------------------------------
# DRAM declarations + host prep + SPMD driver
# ---------------------------------------------------------------------------

def _build_program():
    nc = bass.Bass(trn_type="TRN2")
    dr = {}
    for name, shape, dt in lstm_dram_specs(T):
        dr[name] = nc.dram_tensor(name, shape, dt, kind="ExternalInput")
    for name, shape, dt in CRF_DRAM_SPECS:
        dr[name] = nc.dram_tensor(name, shape, dt, kind="ExternalInput")
    for name, shape, dt in CRF_SCRATCH_SPECS:
        dr[name] = nc.dram_tensor(name, shape, dt)
    dr["scores"] = nc.dram_tensor("scores", [T * 16, 4], F32)
    dr["labels"] = nc.dram_tensor("labels", [NB, T], I32, kind="ExternalOutput")

    with tile.TileContext(nc) as tc:
        with ExitStack() as ctx:
            with ExitStack() as ctx_trans:
                pools = make_pools(ctx, ctx_trans, tc)
                em = LstmEmitter(nc, tc, dr, T, pools)
                em.emit_recurrence()
            with ExitStack() as ctx_crf:
                crf_pool = ctx_crf.enter_context(tc.tile_pool(name="crf", bufs=1))
                emit_crf(nc, tc, dr, crf_pool)
    return nc


_CACHE = {}
LAST_EXEC_NS = None


def kernel(**inputs):
    global LAST_EXEC_NS
    _apply_patches()
    from concourse.bass_utils import run_bass_kernel_spmd

    inp = {k: np.asarray(v) for k, v in inputs.items()}
    if "nc" not in _CACHE:
        _CACHE["nc"] = _build_program()
    nc = _CACHE["nc"]

    shared = host_lstm_shared(inp)

    trans = inp["transitions"].astype(np.float32)
    fromB = inp["from_BOS"].astype(np.float32)
    toEOS = inp["to_EOS"].astype(np.float32)
    b_lab = inp["b_lab"].astype(np.float32)

    pad_seq = inp["pad_seq"].astype(np.int64)
    lens_full = inp["lens"].astype(np.int64)

    in_maps = []
    for core in range(NCORES):
        b0 = core * NB
        seq = pad_seq[b0:b0 + NB]
        lens = lens_full[b0:b0 + NB]
        m = dict(shared)
        m["tokens_f"], m["tokens_b"] = host_tokens(seq, lens, T)
        m.update(host_crf_consts(lens, trans, fromB, toEOS, b_lab))
        in_maps.append(m)

    res = run_bass_kernel_spmd(nc, in_maps, list(range(NCORES)))
    LAST_EXEC_NS = res.exec_time_ns
    out = np.concatenate([res.results[c]["labels"] for c in range(NCORES)], axis=0)
    return out.astype(np.int32)


# revision 15
# speedup vs baseline: 1.1606x; 1.1606x over previous
"""BiLSTM-CRF Trainium2 kernel (Bass/Tile), data-parallel over batch on 8
NeuronCores. Self-contained: host prep + device emission + SPMD runner.

v2 pipeline per core (16 sequences, T=512):
  embedding gather (indirect DMA, fp16) -> DMA XBAR transpose -> Wx matmuls
  (fp16), software-pipelined with TWO independent staggered recurrence
  chains (fwd and bwd), each with a minimal 6-stage serial chain:
    4x Whh matmul (PSUM accum onto ident-copied wx)
    -> one SIGMOID over all 4 gates (tanh(g) = 2*sigma(2g)-1, g-weights
       pre-doubled on host)
    -> V: t = (sig_g - 0.5)*sig_i   [scalar_tensor_tensor]
    -> V: c' = t + m2               [m2 = sig_f*c' computed in the gap;
                                     state c' = c/2]
    -> S: tc = Tanh(c', scale=2)
    -> V: h = sig_o * tc -> hist (fp16)
  The bwd-direction pad masking is free: invalid (t,b) tokens are remapped
  to a synthetic embedding row e* with W_i_b @ e* = -30, so sigma(i)=0
  freezes c=h=0 through the pad prefix.
  Emission scores are interleaved into the second half of the recurrence.
  Then a blocked Viterbi forward scan + blocked backtrace (max-plus /
  one-hot map composition in 32 chunks of 16 steps) as before.
"""
import sys
import types
import numpy as np

import concourse.bass as bass
import concourse.mybir as mybir
from concourse import tile
from concourse.vector_clock import ScopedClock
import bass_rust
from contextlib import ExitStack

F16 = mybir.dt.float16
F32 = mybir.dt.float32
I32 = mybir.dt.int32
AF = mybir.ActivationFunctionType
AX = mybir.AxisListType.X
OP = mybir.AluOpType

B_FULL, T, V, D = 128, 512, 8000, 256
NB = 16          # sequences per core
NCORES = 8


# ---------------------------------------------------------------------------
# Harness workarounds: walrus in this environment accepts only ONE sync-wait
# per instruction; split extras onto NoOps (BIR json pass) and chunk the Tile
# exit drain. Also register the NTFF profile hook shim so BASS_TRACE=1 works.
# ---------------------------------------------------------------------------
import json as _json

_SW_CTR = [0]


def _split_sync_waits(bir_json: bytes) -> bytes:
    d = _json.loads(bir_json)
    changed = False
    for fn in d.get("functions", []):
        for blk in fn.get("blocks", []):
            new_insts = []
            for inst in blk.get("instructions", []):
                si = inst.get("sync_info")
                waits = (si or {}).get("on_wait") or []
                if len(waits) > 1:
                    changed = True
                    for w in waits[:-1]:
                        _SW_CTR[0] += 1
                        nop = {
                            "engine": inst["engine"],
                            "ins": [],
                            "outs": [],
                            "name": f"I-swsplit-{_SW_CTR[0]}",
                            "opcode": "NoOp",
                            "sync_info": {"on_update": [], "on_wait": [w]},
                        }
                        if "debug" in inst:
                            nop["debug"] = inst["debug"]
                        new_insts.append(nop)
                    si["on_wait"] = [waits[-1]]
                new_insts.append(inst)
            blk["instructions"] = new_insts
    return _json.dumps(d).encode() if changed else bir_json


def _patched_drain_and_barrier(self, tick_clock, wait_clock):
    drain_inst = self.nc.sync.drain()
    wait_clock.add_sem_waits(
        drain_inst.ins, ScopedClock({None: tick_clock.global_clock})
    )
    si = drain_inst.ins.sync_info
    if si is not None and si.on_wait is not None and len(si.on_wait) > 1:
        waits = list(si.on_wait)
        drain_inst.ins.sync_info = bass_rust.SyncInfo(
            on_wait=waits[:1], on_update=list(si.on_update or [])
        )
        for i in range(1, len(waits)):
            nop = self.nc.sync.nop()
            nop.ins.sync_info = bass_rust.SyncInfo(on_wait=[waits[i]], on_update=[])
    self.nc.all_engine_barrier()
    assert self.sems is not None
    popped = self.nc._tile_sem_poison_stack.pop()
    assert popped is self._sem_poison
    self.nc.clear_and_free_semaphores(list(self.sems.allocated().values()))
    self.nc.all_engine_barrier()


_PATCHED = [False]


def _apply_patches():
    if _PATCHED[0]:
        return
    _PATCHED[0] = True
    tile.TileContext._drain_and_barrier = _patched_drain_and_barrier
    import concourse.bass_utils as _bu
    import concourse.bass2jax as _b2j

    _orig_compile = _bu.compile_bir_kernel

    def _wrapped(bir_json, tmpdir, neff_name="file.neff"):
        return _orig_compile(_split_sync_waits(bir_json), tmpdir, neff_name)

    _wrapped._swsplit_wrapped = True
    _bu.compile_bir_kernel = _wrapped
    _b2j.compile_bir_kernel = _wrapped

    if "antenv.axon_hooks" not in sys.modules:
        try:
            import trn_agent_boot.trn_boot as _tb
            _hook = _tb._ntff_profile_via_ctypes("/opt/axon/libaxon_pjrt.so")
        except Exception:
            _hook = None
        m = types.ModuleType("antenv.axon_hooks")
        m.get_axon_ntff_profile_hook = lambda: _hook
        m.set_axon_ntff_profile_hook = lambda h: None
        sys.modules["antenv.axon_hooks"] = m


def A(t, off, dims, p0=0):
    # t: pool tile AP [[rowsize, P], [1, rowsize]]. dims[0] is the partition
    # pair whose step is replaced by the tile's canonical per-partition row
    # size; off is the within-partition element offset.
    rs = t.ap[0][0]
    d = [list(x) for x in dims]
    d[0] = [rs, d[0][1]]
    return bass.AP(t.tensor, t.offset + p0 * rs + off, d)


def AD(handle, off, dims):
    return bass.AP(handle, off, [list(d) for d in dims])


def emit_crf(nc, tc, dr, pool):
    """dr: dict of DRAM handles. pool: sbuf tile pool to allocate from."""
    v = nc.vector

    # ---- V0: build T matrices ------------------------------------------
    scT = pool.tile([128, 256], F32)   # (chpos, s, c)
    # scores_dram is tok-major [8192, 4]: addr = (t*16+b)*4 + c
    for g in range(8):
        nc.sync.dma_start(
            A(scT, 0, [[1, 16], [4, 64], [1, 4]], p0=g * 16),
            AD(dr["scores"], g * 4096, [[4, 16], [64, 64], [1, 4]]),
        )
    transb_sb = pool.tile([128, 16], F32)
    nc.sync.dma_start(transb_sb[:], dr["transb16"][None, :].to_broadcast((128, 16)))
    imp_sb = pool.tile([128, 16], F32)
    nc.sync.dma_start(imp_sb[:], dr["impflat"][None, :].to_broadcast((128, 16)))
    vmask_sb = pool.tile([128, 1024], I32)
    nc.sync.dma_start(vmask_sb[:], dr["vmask128"][:])
    fromBp_sb = pool.tile([128, 4], F32)
    nc.sync.dma_start(fromBp_sb[:], dr["fromBp4"][None, :].to_broadcast((128, 4)))

    Traw = pool.tile([128, 1024], F32)  # (chpos, s, p, c)
    v.tensor_add(
        out=A(Traw, 0, [[1, 128], [256, 4], [16, 16], [4, 4], [1, 4]]),
        in0=A(scT, 0, [[1, 128], [64, 4], [4, 16], [0, 4], [1, 4]]),
        in1=A(transb_sb, 0, [[1, 128], [0, 4], [0, 16], [4, 4], [1, 4]]),
    )
    T128 = pool.tile([128, 1024], F32)
    v.select(
        out=A(T128, 0, [[1, 128], [256, 4], [16, 16], [4, 4], [1, 4]]),
        mask=A(vmask_sb, 0, [[1, 128], [256, 4], [16, 16], [4, 4], [1, 4]]),
        on_true=A(Traw, 0, [[1, 128], [256, 4], [16, 16], [4, 4], [1, 4]]),
        on_false=A(imp_sb, 0, [[1, 128], [0, 4], [0, 16], [4, 4], [1, 4]]),
    )
    # step 0 (partitions 0:16, chpos=0, s=0): T = e0 + fromBp (rows equal)
    v.tensor_add(
        out=A(T128, 0, [[1, 16], [4, 4], [1, 4]]),
        in0=A(scT, 0, [[1, 16], [0, 4], [1, 4]]),
        in1=A(fromBp_sb, 0, [[1, 16], [0, 4], [1, 4]]),
    )

    # ---- V1: chunk max-plus products (binary tree, 4 levels) ------------
    # T128 layout (c, s, p, q), col = c*256 + s*16 + p*4 + q. Pairwise
    # max-plus products: P_m = T_{2m} (.) T_{2m+1}, left-assoc equivalent.
    tmp256 = pool.tile([128, 1024], F32)
    TM = pool.tile([128, 2048], F32)
    P1 = pool.tile([128, 512], F32)
    P2 = pool.tile([128, 256], F32)
    P3 = pool.tile([128, 128], F32)
    Ma = pool.tile([128, 64], F32)
    for (src, off, dst, cnt) in (
        (T128, 0, P1, 32), (P1, 0, P2, 16), (P2, 0, P3, 8), (P3, 0, Ma, 4),
    ):
        v.tensor_add(
            out=A(TM, 0, [[1, 128], [64, cnt], [16, 4], [4, 4], [1, 4]]),
            in0=A(src, off, [[1, 128], [32, cnt], [4, 4], [1, 4], [0, 4]]),
            in1=A(src, off + 16, [[1, 128], [32, cnt], [0, 4], [4, 4], [1, 4]]),
        )
        v.tensor_reduce(
            out=A(dst, 0, [[1, 128], [16, cnt], [4, 4], [1, 4]]),
            in_=A(TM, 0, [[1, 128], [64, cnt], [16, 4], [1, 4], [4, 4]]),
            axis=AX, op=OP.max,
        )
    nc.sync.dma_start(
        AD(dr["mdram"], 0, [[64, 128], [1, 64]]),
        A(Ma, 0, [[1, 128], [1, 64]]),
    )

    # ---- V2: serial chunk scan (16 partitions) -------------------------
    M16 = pool.tile([16, 512], F32)
    for g in range(8):
        nc.sync.dma_start(
            A(M16, g * 64, [[1, 16], [1, 64]]),
            AD(dr["mdram"], g * 1024, [[64, 16], [1, 64]]),
        )
    Ball = pool.tile([16, 132], F32)
    v.memset(Ball[:], 0.0)
    t16 = pool.tile([16, 16], F32)
    for c in range(32):
        v.tensor_add(
            out=A(t16, 0, [[1, 16], [4, 4], [1, 4]]),
            in0=A(Ball, c * 4, [[1, 16], [1, 4], [0, 4]]),
            in1=A(M16, c * 16, [[1, 16], [4, 4], [1, 4]]),
        )
        v.tensor_reduce(
            out=A(Ball, (c + 1) * 4, [[1, 16], [1, 4]]),
            in_=A(t16, 0, [[1, 16], [1, 4], [4, 4]]),
            axis=AX, op=OP.max,
        )
    # last label one-hot
    toEOS_sb = pool.tile([16, 4], F32)
    nc.sync.dma_start(toEOS_sb[:], dr["toEOS4"][None, :].to_broadcast((16, 4)))
    c3lab_sb = pool.tile([16, 4], F32)
    nc.sync.dma_start(c3lab_sb[:], dr["c3lab4"][None, :].to_broadcast((16, 4)))
    wiota16 = pool.tile([16, 4], F32)
    nc.sync.dma_start(wiota16[:], dr["wiota4"][None, :].to_broadcast((16, 4)))
    fin = pool.tile([16, 4], F32)
    v.tensor_add(out=fin[:], in0=A(Ball, 128, [[1, 16], [1, 4]]), in1=toEOS_sb[:])
    lmax = pool.tile([16, 1], F32)
    v.tensor_reduce(out=lmax[:], in_=fin[:], axis=AX, op=OP.max)
    loh = pool.tile([16, 4], F32)
    v.tensor_tensor(out=loh[:], in0=fin[:],
                    in1=A(lmax, 0, [[1, 16], [0, 4]]), op=OP.is_equal)
    lohm = pool.tile([16, 4], F32)
    v.tensor_mul(out=lohm[:], in0=loh[:], in1=c3lab_sb[:])
    lenc = pool.tile([16, 1], F32)
    v.tensor_reduce(out=lenc[:], in_=lohm[:], axis=AX, op=OP.max)
    llval = pool.tile([16, 1], F32)
    v.tensor_scalar(out=llval[:], in0=lenc[:], scalar1=-1.0, scalar2=3.0,
                    op0=OP.mult, op1=OP.add)
    lloh = pool.tile([16, 4], F32)
    v.tensor_tensor(out=lloh[:], in0=wiota16[:],
                    in1=A(llval, 0, [[1, 16], [0, 4]]), op=OP.is_equal)
    nc.sync.dma_start(AD(dr["lldram"], 0, [[4, 16], [1, 4]]), lloh[:])
    nc.sync.dma_start(AD(dr["edram"], 0, [[132, 16], [1, 132]]), Ball[:])

    # ---- V3: replay -> backtrace tables --------------------------------
    c3p_sb = pool.tile([128, 16], F32)
    nc.sync.dma_start(c3p_sb[:], dr["c3p16"][None, :].to_broadcast((128, 16)))
    bestA = pool.tile([128, 16], F32)
    bestB = pool.tile([128, 16], F32)
    nc.sync.dma_start(
        bestA[:], AD(dr["edram"], 0, [[16, 8], [132, 16], [4, 4], [1, 4]])
    )
    BT = pool.tile([128, 256], F32)     # (chpos, s, c)
    smat = pool.tile([128, 64], F32)
    oh64 = pool.tile([128, 64], F32)
    enc128 = pool.tile([128, 16], F32)
    bcur, bnxt = bestA, bestB
    for s in range(16):
        v.tensor_add(
            out=A(smat, 0, [[1, 128], [16, 4], [4, 4], [1, 4]]),
            in0=A(bcur, 0, [[1, 128], [4, 4], [1, 4], [0, 4]]),
            in1=A(T128, s * 16, [[1, 128], [256, 4], [4, 4], [1, 4]]),
        )
        v.tensor_reduce(
            out=A(bnxt, 0, [[1, 128], [4, 4], [1, 4]]),
            in_=A(smat, 0, [[1, 128], [16, 4], [1, 4], [4, 4]]),
            axis=AX, op=OP.max,
        )
        v.tensor_tensor(
            out=A(oh64, 0, [[1, 128], [16, 4], [4, 4], [1, 4]]),
            in0=A(smat, 0, [[1, 128], [16, 4], [4, 4], [1, 4]]),
            in1=A(bnxt, 0, [[1, 128], [4, 4], [0, 4], [1, 4]]),
            op=OP.is_equal,
        )
        v.tensor_mul(
            out=A(oh64, 0, [[1, 128], [16, 4], [4, 4], [1, 4]]),
            in0=A(oh64, 0, [[1, 128], [16, 4], [4, 4], [1, 4]]),
            in1=A(c3p_sb, 0, [[1, 128], [0, 4], [4, 4], [1, 4]]),
        )
        v.tensor_reduce(
            out=A(enc128, 0, [[1, 128], [4, 4], [1, 4]]),
            in_=A(oh64, 0, [[1, 128], [16, 4], [1, 4], [4, 4]]),
            axis=AX, op=OP.max,
        )
        v.tensor_scalar(
            out=A(BT, s * 4, [[1, 128], [64, 4], [1, 4]]),
            in0=A(enc128, 0, [[1, 128], [4, 4], [1, 4]]),
            scalar1=-1.0, scalar2=3.0, op0=OP.mult, op1=OP.add,
        )
        bcur, bnxt = bnxt, bcur
    nc.sync.dma_start(
        AD(dr["btdram"], 0, [[256, 128], [1, 256]]),
        A(BT, 0, [[1, 128], [1, 256]]),
    )

    # ---- VA: backtrace map tables + chunk compositions -----------------
    BTS = pool.tile([128, 256], F32)
    # top group's last slot is never used; zero-fill before partial overwrite
    v.memset(A(BTS, 252, [[1, 128], [1, 4]]), 0.0)
    nc.sync.dma_start(
        A(BTS, 0, [[1, 128], [1, 252]]),
        AD(dr["btdram"], 4, [[256, 128], [1, 252]]),
    )
    # last slot of each partition: first bt entry of the next chunk group
    nc.sync.dma_start(
        A(BTS, 252, [[1, 112], [1, 4]]),
        AD(dr["btdram"], 16 * 256, [[256, 112], [1, 4]]),
    )
    meq_sb = pool.tile([128, 64], I32)
    mlt_sb = pool.tile([128, 64], I32)
    nc.sync.dma_start(meq_sb[:], dr["meq128"][:])
    nc.sync.dma_start(mlt_sb[:], dr["mlt128"][:])
    lloh128 = pool.tile([128, 4], F32)
    nc.sync.dma_start(lloh128[:], AD(dr["lldram"], 0, [[0, 8], [4, 16], [1, 4]]))
    i4_sb = pool.tile([128, 16], F32)
    nc.sync.dma_start(i4_sb[:], dr["i4flat"][None, :].to_broadcast((128, 16)))
    wiota128 = pool.tile([128, 4], F32)
    nc.sync.dma_start(wiota128[:], dr["wiota4"][None, :].to_broadcast((128, 4)))

    Fall = pool.tile([128, 1024], F32)  # (chpos, s, u, w)
    tmpA = pool.tile([128, 64], F32)
    for s in range(16):
        # oh(u,w) = bt_{t+1}[u] == w
        v.tensor_tensor(
            out=A(tmpA, 0, [[1, 128], [16, 4], [4, 4], [1, 4]]),
            in0=A(BTS, s * 4, [[1, 128], [64, 4], [1, 4], [0, 4]]),
            in1=A(wiota128, 0, [[1, 128], [0, 4], [0, 4], [1, 4]]),
            op=OP.is_equal,
        )
        # tmp2 = meq ? lloh : I4  ; F = mlt ? oh : tmp2  (write into Fall)
        v.select(
            out=A(Fall, s * 16, [[1, 128], [256, 4], [4, 4], [1, 4]]),
            mask=A(meq_sb, s, [[1, 128], [16, 4], [0, 4], [0, 4]]),
            on_true=A(lloh128, 0, [[1, 128], [0, 4], [0, 4], [1, 4]]),
            on_false=A(i4_sb, 0, [[1, 128], [0, 4], [4, 4], [1, 4]]),
        )
        v.select(
            out=A(Fall, s * 16, [[1, 128], [256, 4], [4, 4], [1, 4]]),
            mask=A(mlt_sb, s, [[1, 128], [16, 4], [0, 4], [0, 4]]),
            on_true=A(tmpA, 0, [[1, 128], [16, 4], [4, 4], [1, 4]]),
            on_false=A(Fall, s * 16, [[1, 128], [256, 4], [4, 4], [1, 4]]),
        )
    # binary tree for G = F15 (x) F14 (x) ... (x) F0 (one-hot max-product);
    # left operand of each pairwise product is the HIGHER index.
    Ga = pool.tile([128, 64], F32)
    for (src, dst, cnt) in (
        (Fall, P1, 32), (P1, P2, 16), (P2, P3, 8), (P3, Ga, 4),
    ):
        v.tensor_mul(
            out=A(TM, 0, [[1, 128], [64, cnt], [16, 4], [4, 4], [1, 4]]),
            in0=A(src, 16, [[1, 128], [32, cnt], [4, 4], [1, 4], [0, 4]]),
            in1=A(src, 0, [[1, 128], [32, cnt], [0, 4], [4, 4], [1, 4]]),
        )
        v.tensor_reduce(
            out=A(dst, 0, [[1, 128], [16, cnt], [4, 4], [1, 4]]),
            in_=A(TM, 0, [[1, 128], [64, cnt], [16, 4], [1, 4], [4, 4]]),
            axis=AX, op=OP.max,
        )
    nc.sync.dma_start(
        AD(dr["gdram"], 0, [[64, 128], [1, 64]]),
        A(Ga, 0, [[1, 128], [1, 64]]),
    )

    # ---- VB: serial reverse chunk scan (16 partitions) -----------------
    Gall16 = pool.tile([16, 512], F32)
    for g in range(8):
        nc.sync.dma_start(
            A(Gall16, g * 64, [[1, 16], [1, 64]]),
            AD(dr["gdram"], g * 1024, [[64, 16], [1, 64]]),
        )
    EB = pool.tile([16, 132], F32)
    nc.sync.dma_start(
        A(EB, 128, [[1, 16], [1, 4]]), dr["e0oh4"][None, :].to_broadcast((16, 4))
    )
    tb16 = pool.tile([16, 16], F32)
    for c in range(31, -1, -1):
        v.tensor_mul(
            out=tb16[:],
            in0=A(EB, (c + 1) * 4, [[1, 16], [1, 4], [0, 4]]),
            in1=A(Gall16, c * 16, [[1, 16], [4, 4], [1, 4]]),
        )
        v.tensor_reduce(
            out=A(EB, c * 4, [[1, 16], [1, 4]]),
            in_=A(tb16, 0, [[1, 16], [1, 4], [4, 4]]),
            axis=AX, op=OP.max,
        )
    nc.sync.dma_start(AD(dr["ebdram"], 0, [[132, 16], [1, 132]]), EB[:])

    # ---- VC: labels -----------------------------------------------------
    cohE = pool.tile([128, 16], F32)
    nc.sync.dma_start(
        cohE[:], AD(dr["ebdram"], 4, [[16, 8], [132, 16], [4, 4], [1, 4]])
    )
    LABOH = pool.tile([128, 256], F32)  # (chpos, s, w)
    tmpc = pool.tile([128, 64], F32)
    for s in range(15, -1, -1):
        if s == 15:
            in0 = A(cohE, 0, [[1, 128], [4, 4], [1, 4], [0, 4]])
        else:
            in0 = A(LABOH, (s + 1) * 4, [[1, 128], [64, 4], [1, 4], [0, 4]])
        v.tensor_mul(
            out=A(tmpc, 0, [[1, 128], [16, 4], [4, 4], [1, 4]]),
            in0=in0,
            in1=A(Fall, s * 16, [[1, 128], [256, 4], [4, 4], [1, 4]]),
        )
        v.tensor_reduce(
            out=A(LABOH, s * 4, [[1, 128], [64, 4], [1, 4]]),
            in_=A(tmpc, 0, [[1, 128], [16, 4], [1, 4], [4, 4]]),
            axis=AX, op=OP.max,
        )
    omask_sb = pool.tile([128, 64], F32)
    nc.sync.dma_start(omask_sb[:], dr["outmask128"][:])
    labv = pool.tile([128, 64], F32)
    tmpl = pool.tile([128, 256], F32)
    v.tensor_mul(
        out=A(tmpl, 0, [[1, 128], [64, 4], [4, 16], [1, 4]]),
        in0=A(LABOH, 0, [[1, 128], [64, 4], [4, 16], [1, 4]]),
        in1=A(wiota128, 0, [[1, 128], [0, 4], [0, 16], [1, 4]]),
    )
    v.tensor_reduce(
        out=A(labv, 0, [[1, 128], [16, 4], [1, 16]]),
        in_=A(tmpl, 0, [[1, 128], [64, 4], [4, 16], [1, 4]]),
        axis=AX, op=OP.add,
    )
    v.tensor_mul(out=labv[:], in0=labv[:], in1=omask_sb[:])
    labi = pool.tile([128, 64], I32)
    v.tensor_copy(labi[:], labv[:])
    for cp in range(4):
        nc.sync.dma_start(
            AD(dr["labels"], 16 * cp, [[64, 8], [512, 16], [1, 16]]),
            A(labi, cp * 16, [[1, 128], [1, 16]]),
        )


def host_crf_consts(lens, trans, fromB, toEOS, b_lab):
    """All host-side constant arrays, keyed to match dram handle names."""
    import numpy as np
    T, B, L = 512, 16, 4
    NEG = -1e9
    out = {}
    out["transb16"] = (trans + b_lab[None, :]).astype(np.float32).reshape(16)
    imp = np.full((L, L), NEG, np.float32)
    np.fill_diagonal(imp, 0.0)
    out["impflat"] = imp.reshape(16)
    out["fromBp4"] = (fromB + b_lab).astype(np.float32)
    out["toEOS4"] = toEOS.astype(np.float32)
    out["c3lab4"] = (3.0 - np.arange(4)).astype(np.float32)
    out["wiota4"] = np.arange(4).astype(np.float32)
    out["c3p16"] = np.repeat(3.0 - np.arange(4), 4).astype(np.float32)
    out["i4flat"] = np.eye(4, dtype=np.float32).reshape(16)
    out["e0oh4"] = np.array([1, 0, 0, 0], np.float32)
    # t value at (P, chpos, s):  P = chgrp*16 + b ; t = 16*(4*chgrp+chpos)+s
    P_chgrp = np.arange(128) // 16
    P_b = np.arange(128) % 16
    chpos = np.arange(4)
    s = np.arange(16)
    tt = 16 * (4 * P_chgrp[:, None, None] + chpos[None, :, None]) + s[None, None, :]
    lb = lens[P_b][:, None, None]
    vm = (tt < lb)
    out["vmask128"] = np.repeat(
        vm.reshape(128, 64)[:, :, None], 16, axis=2
    ).reshape(128, 1024).astype(np.int32)
    out["meq128"] = (tt == lb - 1).reshape(128, 64).astype(np.int32)
    out["mlt128"] = (tt < lb - 1).reshape(128, 64).astype(np.int32)
    out["outmask128"] = (tt < lb).reshape(128, 64).astype(np.float32)
    return out


CRF_DRAM_SPECS = [
    ("transb16", [16], F32), ("impflat", [16], F32), ("fromBp4", [4], F32),
    ("toEOS4", [4], F32), ("c3lab4", [4], F32), ("wiota4", [4], F32),
    ("c3p16", [16], F32), ("i4flat", [16], F32), ("e0oh4", [4], F32),
    ("vmask128", [128, 1024], I32), ("meq128", [128, 64], I32),
    ("mlt128", [128, 64], I32), ("outmask128", [128, 64], F32),
]
CRF_SCRATCH_SPECS = [
    ("mdram", [8192], F32), ("edram", [2112], F32), ("btdram", [32832], F32),
    ("gdram", [8192], F32), ("lldram", [64], F32), ("ebdram", [2112], F32),
]


class LstmEmitter:
    """Two staggered chains (f, b), minimal per-step serial chain."""

    def __init__(self, nc, tc, dr, T, pools):
        self.nc, self.tc, self.dr, self.T = nc, tc, dr, T
        self.NBLK = T // 64
        p = pools
        self.hist = {}
        for d in ("f", "b"):
            h = p["hist"].tile([128, (T + 1) * 16], F16, name=f"hist_{d}")
            self.hist[d] = h
        nc.vector.memset(self.hist["f"][:, 0:16], 0.0)
        nc.vector.memset(self.hist["b"][:, T * 16:(T + 1) * 16], 0.0)
        # true cell state c, fp32, SBUF (DVE/ACT SBUF access is cheapest)
        self.cst = {}
        for d in ("f", "b"):
            c = p["state"].tile([128, 16], F32, name=f"c_{d}")
            nc.vector.memset(c[:], 0.0)
            self.cst[d] = c
        # weights
        self.whhT = {}
        self.wihT = {}
        self.biasT = {}
        self.wlabT = {}
        for d in ("f", "b"):
            w = p["wts"].tile([128, 512], F16, name=f"whh_{d}")
            nc.sync.dma_start(w[:], dr[f"whhT_{d}"][:])
            self.whhT[d] = w
            hs = []
            for h in range(2):
                wh = p["wts"].tile([128, 512], F16, name=f"wih_{d}{h}")
                nc.sync.dma_start(wh[:], dr[f"wihT_{d}{h}"][:])
                hs.append(wh)
            self.wihT[d] = hs
            bt = p["wts"].tile([128, 4], F32, name=f"bias_{d}")
            nc.sync.dma_start(bt[:], dr[f"biasT_{d}"][:])
            self.biasT[d] = bt
            wl = p["wts"].tile([128, 4], F16, name=f"wlab_{d}")
            nc.sync.dma_start(wl[:], dr[f"wlabT_{d}"][:])
            self.wlabT[d] = wl
        self.ident = p["wts"].tile([128, 128], F16)
        nc.sync.dma_start(self.ident[:], dr["ident"][:])
        self.pools = p
        self.wx = {}   # (dir, blk) -> tile [128, 4096] fp16, (j)(tin)(b)

    # ---- production of one dir-block's wx ------------------------------
    def production_items(self, d, blk):
        """Closures emitting gather / PE-transpose / matmul / bias work that
        materialize per-slab wx tiles (slab = 128 tokens = 8 tins). Slabs are
        emitted in chain consumption order (b reversed)."""
        nc, dr, p = self.nc, self.dr, self.pools
        items = []
        state = {}

        def idx_load():
            state["idx"] = p["idx"].tile([128, 8], I32, name=f"idx_{d}")
            nc.sync.dma_start(
                state["idx"][:],
                bass.AP(dr[f"tokens_{d}"], blk * 1024, [[1, 128], [128, 8]]),
            )

        def slab_items(i):
            st = {}

            def gather():
                t = p["xg"].tile([128, 256], F16)
                nc.gpsimd.indirect_dma_start(
                    out=t[:], out_offset=None, in_=dr["emb16"][:],
                    in_offset=bass.IndirectOffsetOnAxis(
                        ap=state["idx"][:, i:i + 1], axis=0),
                )
                st["xg"] = t
                st["xt"] = p["xt"].tile([128, 256], F16, name="xt")
                wxs = p[f"wx_{d}"].tile([128, 512], F16, name=f"wx_{d}")
                st["wx"] = wxs
                self.wx[(d, blk, i)] = wxs

            def transp(h):
                def go():
                    ps = p["tp_ps"].tile([128, 128], F16)
                    nc.tensor.transpose(
                        out=ps[:], in_=st["xg"][:, h * 128:(h + 1) * 128],
                        identity=self.ident[:],
                    )
                    nc.vector.tensor_copy(
                        st["xt"][:, h * 128:(h + 1) * 128], ps[:])
                return go

            def mmevac(j):
                def go():
                    ps = p["wx_ps"].tile([128, 128], F32)
                    for h in range(2):
                        nc.tensor.matmul(
                            out=ps[:],
                            lhsT=self.wihT[d][h][:, j * 128:(j + 1) * 128],
                            rhs=st["xt"][:, h * 128:(h + 1) * 128],
                            start=(h == 0), stop=(h == 1),
                        )
                    dst = st["wx"][:, j * 128:(j + 1) * 128]
                    if j % 2 == 0:
                        nc.scalar.activation(
                            out=dst, in_=ps[:], func=AF.Identity,
                            bias=self.biasT[d][:, j:j + 1])
                    else:
                        nc.vector.tensor_scalar(
                            out=dst, in0=ps[:],
                            scalar1=self.biasT[d][:, j:j + 1], scalar2=None,
                            op0=OP.add)
                return go

            out = [gather, transp(0), transp(1)]
            out += [mmevac(j) for j in range(4)]
            return out

        items.append(idx_load)
        order = range(8) if d == "f" else range(7, -1, -1)
        for i in order:
            items.extend(slab_items(i))
        return items

    # ---- one chain step ------------------------------------------------
    def slot(self, d, t):
        nc, p = self.nc, self.pools
        tins = t % 64
        blk = t // 64
        ha = self.hist[d]
        if d == "f":
            hprev = ha[:, t * 16:(t + 1) * 16]
            hout = ha[:, (t + 1) * 16:(t + 2) * 16]
        else:
            hprev = ha[:, (t + 1) * 16:(t + 2) * 16]
            hout = ha[:, t * 16:(t + 1) * 16]
        wxt = self.wx[(d, blk, tins // 8)]
        gp = p[f"g_ps_{d}"].tile([128, 64], F32, name=f"g_ps_{d}")
        # wx -> PSUM via identity matmul (cols (j,b) for this tin)
        nc.tensor.matmul(
            out=gp[:],
            lhsT=self.ident[:],
            rhs=A(wxt, (tins % 8) * 16, [[1, 128], [128, 4], [1, 16]]),
            start=True, stop=False,
        )
        # hoist j0's weight load so it runs during the h semaphore wait
        nc.tensor.ldweights(self.whhT[d][:, 0:128])
        for j in range(4):
            mm = nc.tensor.matmul(
                out=gp[:, j * 16:(j + 1) * 16],
                lhsT=self.whhT[d][:, j * 128:(j + 1) * 128],
                rhs=hprev, start=False, stop=(j == 3),
            )
            if j == 0:
                try:
                    mm.ins.ldweights = False
                except AttributeError:
                    pass
        # one sigmoid over all gates (g pre-doubled: tanh(g) = 2*sig(2g)-1)
        act = p[f"act_{d}"].tile([128, 64], F16, name=f"act_{d}")
        nc.scalar.activation(out=act[:], in_=gp[:], func=AF.Sigmoid)
        c = self.cst[d]
        # all chain V-ops in scalar_tensor_tensor form (faster DVE path)
        m2 = p[f"m2_{d}"].tile([128, 16], F32, name=f"m2_{d}")
        nc.vector.scalar_tensor_tensor(
            out=m2[:], in0=act[:, 16:32], scalar=0.0, in1=c[:],
            op0=OP.bypass, op1=OP.mult,
        )
        tt = p[f"t_{d}"].tile([128, 16], F32, name=f"t_{d}")
        nc.vector.scalar_tensor_tensor(
            out=tt[:], in0=act[:, 48:64], scalar=0.5, in1=act[:, 0:16],
            op0=OP.subtract, op1=OP.mult,
        )
        # c = 2*t + m2  (true cell state; tanh(g)*sig(i) = 2*t)
        nc.vector.scalar_tensor_tensor(
            out=c[:], in0=tt[:], scalar=2.0, in1=m2[:],
            op0=OP.mult, op1=OP.add,
        )
        tc_ = p[f"tc_{d}"].tile([128, 16], F32, name=f"tc_{d}")
        nc.scalar.activation(out=tc_[:], in_=c[:], func=AF.Tanh)
        nc.vector.scalar_tensor_tensor(
            out=hout, in0=act[:, 32:48], scalar=0.0, in1=tc_[:],
            op0=OP.bypass, op1=OP.mult,
        )

    # ---- emission score chunk n (tokens n*128 .. (n+1)*128) ------------
    def score_chunk(self, n):
        nc, p = self.nc, self.pools
        ps = p["sc_ps"].tile([128, 4], F32)
        nc.tensor.matmul(out=ps[:],
                         lhsT=self.hist["f"][:, 16 + n * 128: 16 + (n + 1) * 128],
                         rhs=self.wlabT["f"][:], start=True, stop=False)
        nc.tensor.matmul(out=ps[:],
                         lhsT=self.hist["b"][:, n * 128:(n + 1) * 128],
                         rhs=self.wlabT["b"][:], start=False, stop=True)
        sb = p["sc_sb"].tile([128, 4], F32)
        nc.vector.tensor_copy(sb[:], ps[:])
        nc.sync.dma_start(
            bass.AP(self.dr["scores"], n * 512, [[4, 128], [1, 4]]), sb[:]
        )

    # ---- full pipelined emission ---------------------------------------
    def emit_recurrence(self):
        T, NBLK = self.T, self.NBLK
        emitted = [False] * (T * 16 // 128)   # score chunks
        nsc = len(emitted)

        def ready_chunks(s):
            out = []
            for n in range(nsc):
                if not emitted[n] and max(8 * n + 7, (T - 1) - 8 * n) <= s:
                    out.append(n)
            return out

        def interleave(fa, fb):
            out = []
            for x, y in zip(fa, fb):
                out.append(x)
                out.append(y)
            return out

        # pair-0 production is dripped just-in-time at the head of block 0;
        # the first slabs (f slab 0, b slab 7) must be fully EMITTED before
        # the first slots that read them.
        pair0 = interleave(self.production_items("f", 0),
                           self.production_items("b", NBLK - 1))
        PRE = 16
        for it in pair0[:PRE]:
            it()
        for blk in range(NBLK):
            todo = []
            if blk == 0:
                todo += pair0
            if blk + 1 < NBLK:
                todo += interleave(self.production_items("f", blk + 1),
                                   self.production_items("b", NBLK - 2 - blk))
            k = PRE if blk == 0 else 0
            n_front = len(pair0) if blk == 0 else 0
            for tin in range(64):
                s = blk * 64 + tin
                self.slot("f", s)
                dp = 2 * tin + 1
                want = max((dp * len(todo)) // 128, min(n_front, PRE + 3 * dp))
                while k < want:
                    todo[k]()
                    k += 1
                self.slot("b", (T - 1) - s)
                dp = 2 * tin + 2
                want = max((dp * len(todo)) // 128, min(n_front, PRE + 3 * dp))
                while k < want:
                    todo[k]()
                    k += 1
                for n in ready_chunks(s)[:2]:
                    self.score_chunk(n)
                    emitted[n] = True
            while k < len(todo):
                todo[k]()
                k += 1
        for n in range(nsc):
            if not emitted[n]:
                self.score_chunk(n)
                emitted[n] = True


def host_lstm_shared(inp):
    """Batch-independent host arrays (weights etc)."""
    shared = {}
    perm = np.concatenate([np.arange(128), 128 + np.arange(128),
                           384 + np.arange(128), 256 + np.arange(128)])
    emb = np.asarray(inp["emb"]).astype(np.float32)
    # synthetic row 8000: W_i_b @ e* = -30 for every i-gate (freezes bwd state
    # through the pad prefix: sigma(i)=0 -> c=h=0)
    W_i_b = np.asarray(inp["W_ih_b"]).astype(np.float64)[0:128]
    e_star, *_ = np.linalg.lstsq(W_i_b, np.full(128, -30.0), rcond=None)
    emb16 = np.concatenate([emb, e_star[None, :].astype(np.float32)], axis=0)
    shared["emb16"] = emb16.astype(np.float16)
    for d, sfx in (("f", "_f"), ("b", "_b")):
        wih = np.asarray(inp[f"W_ih{sfx}"]).astype(np.float32)[perm].copy()
        whh = np.asarray(inp[f"W_hh{sfx}"]).astype(np.float32)[perm].copy()
        bias = (np.asarray(inp[f"b_ih{sfx}"]) +
                np.asarray(inp[f"b_hh{sfx}"])).astype(np.float32)[perm].copy()
        # tanh gate: pre-double (tanh(g) = 2*sigmoid(2g) - 1)
        wih[384:] *= 2.0
        whh[384:] *= 2.0
        bias[384:] *= 2.0
        shared[f"wihT_{d}0"] = np.ascontiguousarray(wih.T[:128]).astype(np.float16)
        shared[f"wihT_{d}1"] = np.ascontiguousarray(wih.T[128:]).astype(np.float16)
        shared[f"whhT_{d}"] = np.ascontiguousarray(whh.T).astype(np.float16)
        shared[f"biasT_{d}"] = np.ascontiguousarray(
            bias.reshape(4, 128).T).astype(np.float32)
        wl = np.asarray(inp["W_lab"]).astype(np.float32)
        half = wl[:, :128] if d == "f" else wl[:, 128:]
        shared[f"wlabT_{d}"] = np.ascontiguousarray(half.T).astype(np.float16)
    shared["ident"] = np.eye(128, dtype=np.float16)
    return shared


def host_tokens(pad_seq, lens, T=512):
    """Per-core token arrays: fwd natural; bwd with pad positions remapped to
    the synthetic frozen-state row (8000)."""
    tok_f = np.ascontiguousarray(pad_seq.T).reshape(-1).astype(np.int32)
    tb = pad_seq.T.copy().astype(np.int32)          # [T, NB]
    invalid = np.arange(T)[:, None] >= lens[None, :]
    tb[invalid] = 8000
    tok_b = np.ascontiguousarray(tb).reshape(-1)
    return tok_f, tok_b


def lstm_dram_specs(T=512):
    return [
        ("emb16", [8001, 256], F16),
        ("tokens_f", [T * 16], I32), ("tokens_b", [T * 16], I32),
        ("wihT_f0", [128, 512], F16), ("wihT_f1", [128, 512], F16),
        ("wihT_b0", [128, 512], F16), ("wihT_b1", [128, 512], F16),
        ("whhT_f", [128, 512], F16), ("whhT_b", [128, 512], F16),
        ("biasT_f", [128, 4], F32), ("biasT_b", [128, 4], F32),
        ("wlabT_f", [128, 4], F16), ("wlabT_b", [128, 4], F16),
        ("ident", [128, 128], F16),
    ]


def make_pools(ctx_persist, ctx_trans, tc):
    p = {}
    p["hist"] = ctx_persist.enter_context(tc.tile_pool(name="hist", bufs=1))
    p["state"] = ctx_persist.enter_context(tc.tile_pool(name="state", bufs=1))
    p["wts"] = ctx_persist.enter_context(tc.tile_pool(name="wts", bufs=1))
    p["idx"] = ctx_trans.enter_context(tc.tile_pool(name="idx", bufs=4))
    p["xg"] = ctx_trans.enter_context(tc.tile_pool(name="xg", bufs=6))
    p["xt"] = ctx_trans.enter_context(tc.tile_pool(name="xt", bufs=4))
    p["wx_f"] = ctx_trans.enter_context(tc.tile_pool(name="wx_f", bufs=16))
    p["wx_b"] = ctx_trans.enter_context(tc.tile_pool(name="wx_b", bufs=16))
    p["tp_ps"] = ctx_trans.enter_context(tc.tile_pool(name="tp_ps", bufs=2, space="PSUM"))
    p["wx_ps"] = ctx_trans.enter_context(tc.tile_pool(name="wx_ps", bufs=1, space="PSUM"))
    p["g_ps_f"] = ctx_trans.enter_context(tc.tile_pool(name="g_ps_f", bufs=2, space="PSUM"))
    p["g_ps_b"] = ctx_trans.enter_context(tc.tile_pool(name="g_ps_b", bufs=2, space="PSUM"))
    p["sc_ps"] = ctx_trans.enter_context(tc.tile_pool(name="sc_ps", bufs=1, space="PSUM"))
    p["sc_sb"] = ctx_trans.enter_context(tc.tile_pool(name="sc_sb", bufs=4))
    for d in ("f", "b"):
        for nm in ("act", "m2", "t", "tc"):
            p[f"{nm}_{d}"] = ctx_trans.enter_context(
                tc.tile_pool(name=f"{nm}_{d}", bufs=2))
    return p


# ---------------------------------------------------------------------------
# DRAM declarations + host prep + SPMD driver
# ---------------------------------------------------------------------------

def _build_program():
    nc = bass.Bass(trn_type="TRN2")
    dr = {}
    for name, shape, dt in lstm_dram_specs(T):
        dr[name] = nc.dram_tensor(name, shape, dt, kind="ExternalInput")
    for name, shape, dt in CRF_DRAM_SPECS:
        dr[name] = nc.dram_tensor(name, shape, dt, kind="ExternalInput")
    for name, shape, dt in CRF_SCRATCH_SPECS:
        dr[name] = nc.dram_tensor(name, shape, dt)
    dr["scores"] = nc.dram_tensor("scores", [T * 16, 4], F32)
    dr["labels"] = nc.dram_tensor("labels", [NB, T], I32, kind="ExternalOutput")

    with tile.TileContext(nc) as tc:
        with ExitStack() as ctx:
            with ExitStack() as ctx_trans:
                pools = make_pools(ctx, ctx_trans, tc)
                em = LstmEmitter(nc, tc, dr, T, pools)
                em.emit_recurrence()
            with ExitStack() as ctx_crf:
                crf_pool = ctx_crf.enter_context(tc.tile_pool(name="crf", bufs=1))
                emit_crf(nc, tc, dr, crf_pool)
    return nc


_CACHE = {}
LAST_EXEC_NS = None


def kernel(**inputs):
    global LAST_EXEC_NS
    _apply_patches()
    from concourse.bass_utils import run_bass_kernel_spmd

    inp = {k: np.asarray(v) for k, v in inputs.items()}
    if "nc" not in _CACHE:
        _CACHE["nc"] = _build_program()
    nc = _CACHE["nc"]

    shared = host_lstm_shared(inp)

    trans = inp["transitions"].astype(np.float32)
    fromB = inp["from_BOS"].astype(np.float32)
    toEOS = inp["to_EOS"].astype(np.float32)
    b_lab = inp["b_lab"].astype(np.float32)

    pad_seq = inp["pad_seq"].astype(np.int64)
    lens_full = inp["lens"].astype(np.int64)

    in_maps = []
    for core in range(NCORES):
        b0 = core * NB
        seq = pad_seq[b0:b0 + NB]
        lens = lens_full[b0:b0 + NB]
        m = dict(shared)
        m["tokens_f"], m["tokens_b"] = host_tokens(seq, lens, T)
        m.update(host_crf_consts(lens, trans, fromB, toEOS, b_lab))
        in_maps.append(m)

    res = run_bass_kernel_spmd(nc, in_maps, list(range(NCORES)))
    LAST_EXEC_NS = res.exec_time_ns
    out = np.concatenate([res.results[c]["labels"] for c in range(NCORES)], axis=0)
    return out.astype(np.int32)


# revision 24
# speedup vs baseline: 1.1790x; 1.0159x over previous
"""BiLSTM-CRF Trainium2 kernel (Bass/Tile), data-parallel over batch on 8
NeuronCores. Self-contained: host prep + device emission + SPMD runner.

v2 pipeline per core (16 sequences, T=512):
  embedding gather (indirect DMA, fp16) -> DMA XBAR transpose -> Wx matmuls
  (fp16), software-pipelined with TWO independent staggered recurrence
  chains (fwd and bwd), each with a minimal 6-stage serial chain:
    4x Whh matmul (PSUM accum onto ident-copied wx)
    -> one SIGMOID over all 4 gates (tanh(g) = 2*sigma(2g)-1, g-weights
       pre-doubled on host)
    -> V: t = (sig_g - 0.5)*sig_i   [scalar_tensor_tensor]
    -> V: c' = t + m2               [m2 = sig_f*c' computed in the gap;
                                     state c' = c/2]
    -> S: tc = Tanh(c', scale=2)
    -> V: h = sig_o * tc -> hist (fp16)
  The bwd-direction pad masking is free: invalid (t,b) tokens are remapped
  to a synthetic embedding row e* with W_i_b @ e* = -30, so sigma(i)=0
  freezes c=h=0 through the pad prefix.
  Emission scores are interleaved into the second half of the recurrence.
  Then a blocked Viterbi forward scan + blocked backtrace (max-plus /
  one-hot map composition in 32 chunks of 16 steps) as before.
"""
import sys
import types
import numpy as np

import concourse.bass as bass
import concourse.mybir as mybir
from concourse import tile
from concourse.vector_clock import ScopedClock
import bass_rust
from contextlib import ExitStack

F16 = mybir.dt.float16
F32 = mybir.dt.float32
I32 = mybir.dt.int32
AF = mybir.ActivationFunctionType
AX = mybir.AxisListType.X
OP = mybir.AluOpType

B_FULL, T, V, D = 128, 512, 8000, 256
NB = 16          # sequences per core
NCORES = 8


# ---------------------------------------------------------------------------
# Harness workarounds: walrus in this environment accepts only ONE sync-wait
# per instruction; split extras onto NoOps (BIR json pass) and chunk the Tile
# exit drain. Also register the NTFF profile hook shim so BASS_TRACE=1 works.
# ---------------------------------------------------------------------------
import json as _json

_SW_CTR = [0]


def _split_sync_waits(bir_json: bytes) -> bytes:
    d = _json.loads(bir_json)
    changed = False
    for fn in d.get("functions", []):
        for blk in fn.get("blocks", []):
            new_insts = []
            for inst in blk.get("instructions", []):
                si = inst.get("sync_info")
                waits = (si or {}).get("on_wait") or []
                if len(waits) > 1:
                    changed = True
                    for w in waits[:-1]:
                        _SW_CTR[0] += 1
                        nop = {
                            "engine": inst["engine"],
                            "ins": [],
                            "outs": [],
                            "name": f"I-swsplit-{_SW_CTR[0]}",
                            "opcode": "NoOp",
                            "sync_info": {"on_update": [], "on_wait": [w]},
                        }
                        if "debug" in inst:
                            nop["debug"] = inst["debug"]
                        new_insts.append(nop)
                    si["on_wait"] = [waits[-1]]
                new_insts.append(inst)
            blk["instructions"] = new_insts
    return _json.dumps(d).encode() if changed else bir_json


def _patched_drain_and_barrier(self, tick_clock, wait_clock):
    drain_inst = self.nc.sync.drain()
    wait_clock.add_sem_waits(
        drain_inst.ins, ScopedClock({None: tick_clock.global_clock})
    )
    si = drain_inst.ins.sync_info
    if si is not None and si.on_wait is not None and len(si.on_wait) > 1:
        waits = list(si.on_wait)
        drain_inst.ins.sync_info = bass_rust.SyncInfo(
            on_wait=waits[:1], on_update=list(si.on_update or [])
        )
        for i in range(1, len(waits)):
            nop = self.nc.sync.nop()
            nop.ins.sync_info = bass_rust.SyncInfo(on_wait=[waits[i]], on_update=[])
    self.nc.all_engine_barrier()
    assert self.sems is not None
    popped = self.nc._tile_sem_poison_stack.pop()
    assert popped is self._sem_poison
    self.nc.clear_and_free_semaphores(list(self.sems.allocated().values()))
    self.nc.all_engine_barrier()


_PATCHED = [False]


def _apply_patches():
    if _PATCHED[0]:
        return
    _PATCHED[0] = True
    tile.TileContext._drain_and_barrier = _patched_drain_and_barrier
    import concourse.bass_utils as _bu
    import concourse.bass2jax as _b2j

    _orig_compile = _bu.compile_bir_kernel

    def _wrapped(bir_json, tmpdir, neff_name="file.neff"):
        return _orig_compile(_split_sync_waits(bir_json), tmpdir, neff_name)

    _wrapped._swsplit_wrapped = True
    _bu.compile_bir_kernel = _wrapped
    _b2j.compile_bir_kernel = _wrapped

    if "antenv.axon_hooks" not in sys.modules:
        try:
            import trn_agent_boot.trn_boot as _tb
            _hook = _tb._ntff_profile_via_ctypes("/opt/axon/libaxon_pjrt.so")
        except Exception:
            _hook = None
        m = types.ModuleType("antenv.axon_hooks")
        m.get_axon_ntff_profile_hook = lambda: _hook
        m.set_axon_ntff_profile_hook = lambda h: None
        sys.modules["antenv.axon_hooks"] = m


def A(t, off, dims, p0=0):
    # t: pool tile AP [[rowsize, P], [1, rowsize]]. dims[0] is the partition
    # pair whose step is replaced by the tile's canonical per-partition row
    # size; off is the within-partition element offset.
    rs = t.ap[0][0]
    d = [list(x) for x in dims]
    d[0] = [rs, d[0][1]]
    return bass.AP(t.tensor, t.offset + p0 * rs + off, d)


def AD(handle, off, dims):
    return bass.AP(handle, off, [list(d) for d in dims])


def emit_crf(nc, tc, dr, pool):
    """dr: dict of DRAM handles. pool: sbuf tile pool to allocate from."""
    v = nc.vector

    # ---- V0: build T matrices ------------------------------------------
    scT = pool.tile([128, 256], F32)   # (chpos, s, c)
    # scores_dram is tok-major [8192, 4]: addr = (t*16+b)*4 + c
    for g in range(8):
        nc.sync.dma_start(
            A(scT, 0, [[1, 16], [4, 64], [1, 4]], p0=g * 16),
            AD(dr["scores"], g * 4096, [[4, 16], [64, 64], [1, 4]]),
        )
    transb_sb = pool.tile([128, 16], F32)
    nc.sync.dma_start(transb_sb[:], dr["transb16"][None, :].to_broadcast((128, 16)))
    imp_sb = pool.tile([128, 16], F32)
    nc.sync.dma_start(imp_sb[:], dr["impflat"][None, :].to_broadcast((128, 16)))
    vmask_sb = pool.tile([128, 1024], I32)
    nc.sync.dma_start(vmask_sb[:], dr["vmask128"][:])
    fromBp_sb = pool.tile([128, 4], F32)
    nc.sync.dma_start(fromBp_sb[:], dr["fromBp4"][None, :].to_broadcast((128, 4)))

    Traw = pool.tile([128, 1024], F32)  # (chpos, s, p, c)
    v.tensor_add(
        out=A(Traw, 0, [[1, 128], [256, 4], [16, 16], [4, 4], [1, 4]]),
        in0=A(scT, 0, [[1, 128], [64, 4], [4, 16], [0, 4], [1, 4]]),
        in1=A(transb_sb, 0, [[1, 128], [0, 4], [0, 16], [4, 4], [1, 4]]),
    )
    T128 = pool.tile([128, 1024], F32)
    v.select(
        out=A(T128, 0, [[1, 128], [256, 4], [16, 16], [4, 4], [1, 4]]),
        mask=A(vmask_sb, 0, [[1, 128], [256, 4], [16, 16], [4, 4], [1, 4]]),
        on_true=A(Traw, 0, [[1, 128], [256, 4], [16, 16], [4, 4], [1, 4]]),
        on_false=A(imp_sb, 0, [[1, 128], [0, 4], [0, 16], [4, 4], [1, 4]]),
    )
    # step 0 (partitions 0:16, chpos=0, s=0): T = e0 + fromBp (rows equal)
    v.tensor_add(
        out=A(T128, 0, [[1, 16], [4, 4], [1, 4]]),
        in0=A(scT, 0, [[1, 16], [0, 4], [1, 4]]),
        in1=A(fromBp_sb, 0, [[1, 16], [0, 4], [1, 4]]),
    )

    # ---- V1: chunk max-plus products (binary tree, 4 levels) ------------
    # T128 layout (c, s, p, q), col = c*256 + s*16 + p*4 + q. Pairwise
    # max-plus products: P_m = T_{2m} (.) T_{2m+1}, left-assoc equivalent.
    tmp256 = pool.tile([128, 1024], F32)
    TM = pool.tile([128, 2048], F32)
    P1 = pool.tile([128, 512], F32)
    P2 = pool.tile([128, 256], F32)
    P3 = pool.tile([128, 128], F32)
    Ma = pool.tile([128, 64], F32)
    for (src, off, dst, cnt) in (
        (T128, 0, P1, 32), (P1, 0, P2, 16), (P2, 0, P3, 8), (P3, 0, Ma, 4),
    ):
        v.tensor_add(
            out=A(TM, 0, [[1, 128], [64, cnt], [16, 4], [4, 4], [1, 4]]),
            in0=A(src, off, [[1, 128], [32, cnt], [4, 4], [1, 4], [0, 4]]),
            in1=A(src, off + 16, [[1, 128], [32, cnt], [0, 4], [4, 4], [1, 4]]),
        )
        v.tensor_reduce(
            out=A(dst, 0, [[1, 128], [16, cnt], [4, 4], [1, 4]]),
            in_=A(TM, 0, [[1, 128], [64, cnt], [16, 4], [1, 4], [4, 4]]),
            axis=AX, op=OP.max,
        )
    nc.sync.dma_start(
        AD(dr["mdram"], 0, [[64, 128], [1, 64]]),
        A(Ma, 0, [[1, 128], [1, 64]]),
    )

    # ---- V2: serial chunk scan (16 partitions) -------------------------
    M16 = pool.tile([16, 512], F32)
    for g in range(8):
        nc.sync.dma_start(
            A(M16, g * 64, [[1, 16], [1, 64]]),
            AD(dr["mdram"], g * 1024, [[64, 16], [1, 64]]),
        )
    Ball = pool.tile([16, 132], F32)
    v.memset(Ball[:], 0.0)
    t16 = pool.tile([16, 16], F32)
    for c in range(32):
        v.tensor_add(
            out=A(t16, 0, [[1, 16], [4, 4], [1, 4]]),
            in0=A(Ball, c * 4, [[1, 16], [1, 4], [0, 4]]),
            in1=A(M16, c * 16, [[1, 16], [4, 4], [1, 4]]),
        )
        v.tensor_reduce(
            out=A(Ball, (c + 1) * 4, [[1, 16], [1, 4]]),
            in_=A(t16, 0, [[1, 16], [1, 4], [4, 4]]),
            axis=AX, op=OP.max,
        )
    # last label one-hot
    toEOS_sb = pool.tile([16, 4], F32)
    nc.sync.dma_start(toEOS_sb[:], dr["toEOS4"][None, :].to_broadcast((16, 4)))
    c3lab_sb = pool.tile([16, 4], F32)
    nc.sync.dma_start(c3lab_sb[:], dr["c3lab4"][None, :].to_broadcast((16, 4)))
    wiota16 = pool.tile([16, 4], F32)
    nc.sync.dma_start(wiota16[:], dr["wiota4"][None, :].to_broadcast((16, 4)))
    fin = pool.tile([16, 4], F32)
    v.tensor_add(out=fin[:], in0=A(Ball, 128, [[1, 16], [1, 4]]), in1=toEOS_sb[:])
    lmax = pool.tile([16, 1], F32)
    v.tensor_reduce(out=lmax[:], in_=fin[:], axis=AX, op=OP.max)
    loh = pool.tile([16, 4], F32)
    v.tensor_tensor(out=loh[:], in0=fin[:],
                    in1=A(lmax, 0, [[1, 16], [0, 4]]), op=OP.is_equal)
    lohm = pool.tile([16, 4], F32)
    v.tensor_mul(out=lohm[:], in0=loh[:], in1=c3lab_sb[:])
    lenc = pool.tile([16, 1], F32)
    v.tensor_reduce(out=lenc[:], in_=lohm[:], axis=AX, op=OP.max)
    llval = pool.tile([16, 1], F32)
    v.tensor_scalar(out=llval[:], in0=lenc[:], scalar1=-1.0, scalar2=3.0,
                    op0=OP.mult, op1=OP.add)
    lloh = pool.tile([16, 4], F32)
    v.tensor_tensor(out=lloh[:], in0=wiota16[:],
                    in1=A(llval, 0, [[1, 16], [0, 4]]), op=OP.is_equal)
    nc.sync.dma_start(AD(dr["lldram"], 0, [[4, 16], [1, 4]]), lloh[:])
    nc.sync.dma_start(AD(dr["edram"], 0, [[132, 16], [1, 132]]), Ball[:])

    # ---- V3: replay -> backtrace tables --------------------------------
    c3p_sb = pool.tile([128, 16], F32)
    nc.sync.dma_start(c3p_sb[:], dr["c3p16"][None, :].to_broadcast((128, 16)))
    bestA = pool.tile([128, 16], F32)
    bestB = pool.tile([128, 16], F32)
    nc.sync.dma_start(
        bestA[:], AD(dr["edram"], 0, [[16, 8], [132, 16], [4, 4], [1, 4]])
    )
    BT = pool.tile([128, 256], F32)     # (chpos, s, c)
    smat = pool.tile([128, 64], F32)
    oh64 = pool.tile([128, 64], F32)
    enc128 = pool.tile([128, 16], F32)
    bcur, bnxt = bestA, bestB
    for s in range(16):
        v.tensor_add(
            out=A(smat, 0, [[1, 128], [16, 4], [4, 4], [1, 4]]),
            in0=A(bcur, 0, [[1, 128], [4, 4], [1, 4], [0, 4]]),
            in1=A(T128, s * 16, [[1, 128], [256, 4], [4, 4], [1, 4]]),
        )
        v.tensor_reduce(
            out=A(bnxt, 0, [[1, 128], [4, 4], [1, 4]]),
            in_=A(smat, 0, [[1, 128], [16, 4], [1, 4], [4, 4]]),
            axis=AX, op=OP.max,
        )
        v.tensor_tensor(
            out=A(oh64, 0, [[1, 128], [16, 4], [4, 4], [1, 4]]),
            in0=A(smat, 0, [[1, 128], [16, 4], [4, 4], [1, 4]]),
            in1=A(bnxt, 0, [[1, 128], [4, 4], [0, 4], [1, 4]]),
            op=OP.is_equal,
        )
        v.tensor_mul(
            out=A(oh64, 0, [[1, 128], [16, 4], [4, 4], [1, 4]]),
            in0=A(oh64, 0, [[1, 128], [16, 4], [4, 4], [1, 4]]),
            in1=A(c3p_sb, 0, [[1, 128], [0, 4], [4, 4], [1, 4]]),
        )
        v.tensor_reduce(
            out=A(enc128, 0, [[1, 128], [4, 4], [1, 4]]),
            in_=A(oh64, 0, [[1, 128], [16, 4], [1, 4], [4, 4]]),
            axis=AX, op=OP.max,
        )
        v.tensor_scalar(
            out=A(BT, s * 4, [[1, 128], [64, 4], [1, 4]]),
            in0=A(enc128, 0, [[1, 128], [4, 4], [1, 4]]),
            scalar1=-1.0, scalar2=3.0, op0=OP.mult, op1=OP.add,
        )
        bcur, bnxt = bnxt, bcur
    nc.sync.dma_start(
        AD(dr["btdram"], 0, [[256, 128], [1, 256]]),
        A(BT, 0, [[1, 128], [1, 256]]),
    )

    # ---- VA: backtrace map tables + chunk compositions -----------------
    BTS = pool.tile([128, 256], F32)
    # top group's last slot is never used; zero-fill before partial overwrite
    v.memset(A(BTS, 252, [[1, 128], [1, 4]]), 0.0)
    nc.sync.dma_start(
        A(BTS, 0, [[1, 128], [1, 252]]),
        AD(dr["btdram"], 4, [[256, 128], [1, 252]]),
    )
    # last slot of each partition: first bt entry of the next chunk group
    nc.sync.dma_start(
        A(BTS, 252, [[1, 112], [1, 4]]),
        AD(dr["btdram"], 16 * 256, [[256, 112], [1, 4]]),
    )
    meq_sb = pool.tile([128, 64], I32)
    mlt_sb = pool.tile([128, 64], I32)
    nc.sync.dma_start(meq_sb[:], dr["meq128"][:])
    nc.sync.dma_start(mlt_sb[:], dr["mlt128"][:])
    lloh128 = pool.tile([128, 4], F32)
    nc.sync.dma_start(lloh128[:], AD(dr["lldram"], 0, [[0, 8], [4, 16], [1, 4]]))
    i4_sb = pool.tile([128, 16], F32)
    nc.sync.dma_start(i4_sb[:], dr["i4flat"][None, :].to_broadcast((128, 16)))
    wiota128 = pool.tile([128, 4], F32)
    nc.sync.dma_start(wiota128[:], dr["wiota4"][None, :].to_broadcast((128, 4)))

    Fall = pool.tile([128, 1024], F32)  # (chpos, s, u, w)
    tmpA = pool.tile([128, 64], F32)
    for s in range(16):
        # oh(u,w) = bt_{t+1}[u] == w
        v.tensor_tensor(
            out=A(tmpA, 0, [[1, 128], [16, 4], [4, 4], [1, 4]]),
            in0=A(BTS, s * 4, [[1, 128], [64, 4], [1, 4], [0, 4]]),
            in1=A(wiota128, 0, [[1, 128], [0, 4], [0, 4], [1, 4]]),
            op=OP.is_equal,
        )
        # tmp2 = meq ? lloh : I4  ; F = mlt ? oh : tmp2  (write into Fall)
        v.select(
            out=A(Fall, s * 16, [[1, 128], [256, 4], [4, 4], [1, 4]]),
            mask=A(meq_sb, s, [[1, 128], [16, 4], [0, 4], [0, 4]]),
            on_true=A(lloh128, 0, [[1, 128], [0, 4], [0, 4], [1, 4]]),
            on_false=A(i4_sb, 0, [[1, 128], [0, 4], [4, 4], [1, 4]]),
        )
        v.select(
            out=A(Fall, s * 16, [[1, 128], [256, 4], [4, 4], [1, 4]]),
            mask=A(mlt_sb, s, [[1, 128], [16, 4], [0, 4], [0, 4]]),
            on_true=A(tmpA, 0, [[1, 128], [16, 4], [4, 4], [1, 4]]),
            on_false=A(Fall, s * 16, [[1, 128], [256, 4], [4, 4], [1, 4]]),
        )
    # binary tree for G = F15 (x) F14 (x) ... (x) F0 (one-hot max-product);
    # left operand of each pairwise product is the HIGHER index.
    Ga = pool.tile([128, 64], F32)
    for (src, dst, cnt) in (
        (Fall, P1, 32), (P1, P2, 16), (P2, P3, 8), (P3, Ga, 4),
    ):
        v.tensor_mul(
            out=A(TM, 0, [[1, 128], [64, cnt], [16, 4], [4, 4], [1, 4]]),
            in0=A(src, 16, [[1, 128], [32, cnt], [4, 4], [1, 4], [0, 4]]),
            in1=A(src, 0, [[1, 128], [32, cnt], [0, 4], [4, 4], [1, 4]]),
        )
        v.tensor_reduce(
            out=A(dst, 0, [[1, 128], [16, cnt], [4, 4], [1, 4]]),
            in_=A(TM, 0, [[1, 128], [64, cnt], [16, 4], [1, 4], [4, 4]]),
            axis=AX, op=OP.max,
        )
    nc.sync.dma_start(
        AD(dr["gdram"], 0, [[64, 128], [1, 64]]),
        A(Ga, 0, [[1, 128], [1, 64]]),
    )

    # ---- VB: serial reverse chunk scan (16 partitions) -----------------
    Gall16 = pool.tile([16, 512], F32)
    for g in range(8):
        nc.sync.dma_start(
            A(Gall16, g * 64, [[1, 16], [1, 64]]),
            AD(dr["gdram"], g * 1024, [[64, 16], [1, 64]]),
        )
    EB = pool.tile([16, 132], F32)
    nc.sync.dma_start(
        A(EB, 128, [[1, 16], [1, 4]]), dr["e0oh4"][None, :].to_broadcast((16, 4))
    )
    tb16 = pool.tile([16, 16], F32)
    for c in range(31, -1, -1):
        v.tensor_mul(
            out=tb16[:],
            in0=A(EB, (c + 1) * 4, [[1, 16], [1, 4], [0, 4]]),
            in1=A(Gall16, c * 16, [[1, 16], [4, 4], [1, 4]]),
        )
        v.tensor_reduce(
            out=A(EB, c * 4, [[1, 16], [1, 4]]),
            in_=A(tb16, 0, [[1, 16], [1, 4], [4, 4]]),
            axis=AX, op=OP.max,
        )
    nc.sync.dma_start(AD(dr["ebdram"], 0, [[132, 16], [1, 132]]), EB[:])

    # ---- VC: labels -----------------------------------------------------
    cohE = pool.tile([128, 16], F32)
    nc.sync.dma_start(
        cohE[:], AD(dr["ebdram"], 4, [[16, 8], [132, 16], [4, 4], [1, 4]])
    )
    LABOH = pool.tile([128, 256], F32)  # (chpos, s, w)
    tmpc = pool.tile([128, 64], F32)
    for s in range(15, -1, -1):
        if s == 15:
            in0 = A(cohE, 0, [[1, 128], [4, 4], [1, 4], [0, 4]])
        else:
            in0 = A(LABOH, (s + 1) * 4, [[1, 128], [64, 4], [1, 4], [0, 4]])
        v.tensor_mul(
            out=A(tmpc, 0, [[1, 128], [16, 4], [4, 4], [1, 4]]),
            in0=in0,
            in1=A(Fall, s * 16, [[1, 128], [256, 4], [4, 4], [1, 4]]),
        )
        v.tensor_reduce(
            out=A(LABOH, s * 4, [[1, 128], [64, 4], [1, 4]]),
            in_=A(tmpc, 0, [[1, 128], [16, 4], [1, 4], [4, 4]]),
            axis=AX, op=OP.max,
        )
    omask_sb = pool.tile([128, 64], F32)
    nc.sync.dma_start(omask_sb[:], dr["outmask128"][:])
    labv = pool.tile([128, 64], F32)
    tmpl = pool.tile([128, 256], F32)
    v.tensor_mul(
        out=A(tmpl, 0, [[1, 128], [64, 4], [4, 16], [1, 4]]),
        in0=A(LABOH, 0, [[1, 128], [64, 4], [4, 16], [1, 4]]),
        in1=A(wiota128, 0, [[1, 128], [0, 4], [0, 16], [1, 4]]),
    )
    v.tensor_reduce(
        out=A(labv, 0, [[1, 128], [16, 4], [1, 16]]),
        in_=A(tmpl, 0, [[1, 128], [64, 4], [4, 16], [1, 4]]),
        axis=AX, op=OP.add,
    )
    v.tensor_mul(out=labv[:], in0=labv[:], in1=omask_sb[:])
    labi = pool.tile([128, 64], I32)
    v.tensor_copy(labi[:], labv[:])
    for cp in range(4):
        nc.sync.dma_start(
            AD(dr["labels"], 16 * cp, [[64, 8], [512, 16], [1, 16]]),
            A(labi, cp * 16, [[1, 128], [1, 16]]),
        )


def host_crf_consts(lens, trans, fromB, toEOS, b_lab):
    """All host-side constant arrays, keyed to match dram handle names."""
    import numpy as np
    T, B, L = 512, 16, 4
    NEG = -1e9
    out = {}
    out["transb16"] = (trans + b_lab[None, :]).astype(np.float32).reshape(16)
    imp = np.full((L, L), NEG, np.float32)
    np.fill_diagonal(imp, 0.0)
    out["impflat"] = imp.reshape(16)
    out["fromBp4"] = (fromB + b_lab).astype(np.float32)
    out["toEOS4"] = toEOS.astype(np.float32)
    out["c3lab4"] = (3.0 - np.arange(4)).astype(np.float32)
    out["wiota4"] = np.arange(4).astype(np.float32)
    out["c3p16"] = np.repeat(3.0 - np.arange(4), 4).astype(np.float32)
    out["i4flat"] = np.eye(4, dtype=np.float32).reshape(16)
    out["e0oh4"] = np.array([1, 0, 0, 0], np.float32)
    # t value at (P, chpos, s):  P = chgrp*16 + b ; t = 16*(4*chgrp+chpos)+s
    P_chgrp = np.arange(128) // 16
    P_b = np.arange(128) % 16
    chpos = np.arange(4)
    s = np.arange(16)
    tt = 16 * (4 * P_chgrp[:, None, None] + chpos[None, :, None]) + s[None, None, :]
    lb = lens[P_b][:, None, None]
    vm = (tt < lb)
    out["vmask128"] = np.repeat(
        vm.reshape(128, 64)[:, :, None], 16, axis=2
    ).reshape(128, 1024).astype(np.int32)
    out["meq128"] = (tt == lb - 1).reshape(128, 64).astype(np.int32)
    out["mlt128"] = (tt < lb - 1).reshape(128, 64).astype(np.int32)
    out["outmask128"] = (tt < lb).reshape(128, 64).astype(np.float32)
    return out


CRF_DRAM_SPECS = [
    ("transb16", [16], F32), ("impflat", [16], F32), ("fromBp4", [4], F32),
    ("toEOS4", [4], F32), ("c3lab4", [4], F32), ("wiota4", [4], F32),
    ("c3p16", [16], F32), ("i4flat", [16], F32), ("e0oh4", [4], F32),
    ("vmask128", [128, 1024], I32), ("meq128", [128, 64], I32),
    ("mlt128", [128, 64], I32), ("outmask128", [128, 64], F32),
]
CRF_SCRATCH_SPECS = [
    ("mdram", [8192], F32), ("edram", [2112], F32), ("btdram", [32832], F32),
    ("gdram", [8192], F32), ("lldram", [64], F32), ("ebdram", [2112], F32),
]


class LstmEmitter:
    """Two staggered chains (f, b), minimal per-step serial chain."""

    def __init__(self, nc, tc, dr, T, pools):
        self.nc, self.tc, self.dr, self.T = nc, tc, dr, T
        self.NBLK = T // 64
        p = pools
        self.hist = {}
        for d in ("f", "b"):
            h = p["hist"].tile([128, (T + 1) * 16], F16, name=f"hist_{d}")
            self.hist[d] = h
        nc.vector.memset(self.hist["f"][:, 0:16], 0.0)
        nc.vector.memset(self.hist["b"][:, T * 16:(T + 1) * 16], 0.0)
        # true cell state c, fp32, SBUF (DVE/ACT SBUF access is cheapest)
        self.cst = {}
        for d in ("f", "b"):
            c = p["state"].tile([128, 16], F32, name=f"c_{d}")
            nc.vector.memset(c[:], 0.0)
            self.cst[d] = c
        # weights
        self.whhT = {}
        self.wihT = {}
        self.biasT = {}
        self.wlabT = {}
        for d in ("f", "b"):
            w = p["wts"].tile([128, 512], F16, name=f"whh_{d}")
            nc.sync.dma_start(w[:], dr[f"whhT_{d}"][:])
            self.whhT[d] = w
            hs = []
            for h in range(2):
                wh = p["wts"].tile([128, 512], F16, name=f"wih_{d}{h}")
                nc.sync.dma_start(wh[:], dr[f"wihT_{d}{h}"][:])
                hs.append(wh)
            self.wihT[d] = hs
            bk = p["wts"].tile([4, 128], F16, name=f"biasK4_{d}")
            nc.sync.dma_start(bk[:], dr[f"biasK4_{d}"][:])
            self.biasT[d] = bk
            wl = p["wts"].tile([128, 4], F16, name=f"wlab_{d}")
            nc.sync.dma_start(wl[:], dr[f"wlabT_{d}"][:])
            self.wlabT[d] = wl
        self.biasK4 = self.biasT
        self.ones4 = p["wts"].tile([4, 512], F16, name="ones4")
        nc.sync.dma_start(self.ones4[:], dr["ones4"][:])
        self.ident = p["wts"].tile([128, 128], F16)
        nc.sync.dma_start(self.ident[:], dr["ident"][:])
        self.pools = p
        self.gbank = {}   # (dir, blk, slab) -> PSUM bank tile [128, 512]

    # ---- production of one dir-block's gate banks ----------------------
    def production_items(self, d, blk):
        """Returns (idx_item, gts, mms): gts[p] = [gather, transp, transp]
        and mms[p] = [wxmm, wxmm, biasmm] for consumption position p (the
        p-th slab the chain will consume; b consumes slabs reversed).
        Gates(+bias) are written DIRECTLY into per-slab PSUM banks, layout
        (tin 8)(j 4)(b 16); the recurrence's Whh matmuls accumulate in
        place."""
        nc, dr, p = self.nc, self.dr, self.pools
        state = {}

        def idx_load():
            state["idx"] = p["idx"].tile([128, 8], I32, name=f"idx_{d}")
            nc.sync.dma_start(
                state["idx"][:],
                bass.AP(dr[f"tokens_{d}"], blk * 1024, [[1, 128], [128, 8]]),
            )

        def slab_items(i):
            st = {}

            def gather():
                t = p["xg"].tile([128, 256], F16)
                nc.gpsimd.indirect_dma_start(
                    out=t[:], out_offset=None, in_=dr["emb16"][:],
                    in_offset=bass.IndirectOffsetOnAxis(
                        ap=state["idx"][:, i:i + 1], axis=0),
                )
                st["xg"] = t
                st["xt"] = p["xt"].tile([128, 256], F16, name="xt")

            def transp(h):
                def go():
                    ps = p["tp_ps"].tile([128, 128], F16)
                    nc.tensor.transpose(
                        out=ps[:], in_=st["xg"][:, h * 128:(h + 1) * 128],
                        identity=self.ident[:],
                    )
                    nc.vector.tensor_copy(
                        st["xt"][:, h * 128:(h + 1) * 128], ps[:])
                return go

            def wxmm(jj):
                def go():
                    if jj == 0:
                        st["gb"] = p[f"gb_{d}"].tile(
                            [128, 512], F32, name=f"gb_{d}")
                        self.gbank[(d, blk, i)] = st["gb"]
                    gb = st["gb"]
                    for j in (jj * 2, jj * 2 + 1):
                        dst = A(gb, j * 16, [[1, 128], [64, 8], [1, 16]])
                        for h in range(2):
                            nc.tensor.matmul(
                                out=dst,
                                lhsT=self.wihT[d][h][:, j * 128:(j + 1) * 128],
                                rhs=st["xt"][:, h * 128:(h + 1) * 128],
                                start=(h == 0), stop=False,
                                skip_group_check=True,
                            )
                return go

            def biasmm():
                # bias over the whole bank: K=4 matmul with 0/1 selector rhs
                nc.tensor.matmul(
                    out=st["gb"][:],
                    lhsT=self.biasK4[d][:],
                    rhs=self.ones4[:],
                    start=False, stop=False, skip_group_check=True,
                )
            return [gather, transp(0), transp(1)], [wxmm(0), wxmm(1), biasmm]

        gts, mms = [], []
        order = range(8) if d == "f" else range(7, -1, -1)
        for i in order:
            g, m = slab_items(i)
            gts.append(g)
            mms.append(m)
        return idx_load, gts, mms

    # ---- one chain step ------------------------------------------------
    def slot(self, d, t):
        nc, p = self.nc, self.pools
        tins = t % 64
        blk = t // 64
        ha = self.hist[d]
        if d == "f":
            hprev = ha[:, t * 16:(t + 1) * 16]
            hout = ha[:, (t + 1) * 16:(t + 2) * 16]
        else:
            hprev = ha[:, (t + 1) * 16:(t + 2) * 16]
            hout = ha[:, t * 16:(t + 1) * 16]
        gb = self.gbank[(d, blk, tins // 8)]
        goff = (tins % 8) * 64
        for j in range(4):
            nc.tensor.matmul(
                out=gb[:, goff + j * 16:goff + (j + 1) * 16],
                lhsT=self.whhT[d][:, j * 128:(j + 1) * 128],
                rhs=hprev, start=False, stop=(j == 3),
                skip_group_check=True,
            )
        # one sigmoid over all gates (g pre-doubled: tanh(g) = 2*sig(2g)-1)
        act = p[f"act_{d}"].tile([128, 64], F16, name=f"act_{d}")
        nc.scalar.activation(out=act[:], in_=gb[:, goff:goff + 64],
                             func=AF.Sigmoid)
        c = self.cst[d]
        # all chain V-ops in scalar_tensor_tensor form (faster DVE path)
        m2 = p[f"m2_{d}"].tile([128, 16], F32, name=f"m2_{d}")
        nc.vector.scalar_tensor_tensor(
            out=m2[:], in0=act[:, 16:32], scalar=0.0, in1=c[:],
            op0=OP.bypass, op1=OP.mult,
        )
        tt = p[f"t_{d}"].tile([128, 16], F32, name=f"t_{d}")
        nc.vector.scalar_tensor_tensor(
            out=tt[:], in0=act[:, 48:64], scalar=0.5, in1=act[:, 0:16],
            op0=OP.subtract, op1=OP.mult,
        )
        # c = 2*t + m2  (true cell state; tanh(g)*sig(i) = 2*t)
        nc.vector.scalar_tensor_tensor(
            out=c[:], in0=tt[:], scalar=2.0, in1=m2[:],
            op0=OP.mult, op1=OP.add,
        )
        tc_ = p[f"tc_{d}"].tile([128, 16], F32, name=f"tc_{d}")
        nc.scalar.activation(out=tc_[:], in_=c[:], func=AF.Tanh)
        nc.vector.scalar_tensor_tensor(
            out=hout, in0=act[:, 32:48], scalar=0.0, in1=tc_[:],
            op0=OP.bypass, op1=OP.mult,
        )

    # ---- emission score chunk n (tokens n*128 .. (n+1)*128) ------------
    def score_chunk(self, n):
        nc, p = self.nc, self.pools
        ps = p["sc_ps"].tile([128, 4], F32)
        nc.tensor.matmul(out=ps[:],
                         lhsT=self.hist["f"][:, 16 + n * 128: 16 + (n + 1) * 128],
                         rhs=self.wlabT["f"][:], start=True, stop=False)
        nc.tensor.matmul(out=ps[:],
                         lhsT=self.hist["b"][:, n * 128:(n + 1) * 128],
                         rhs=self.wlabT["b"][:], start=False, stop=True)
        sb = p["sc_sb"].tile([128, 4], F32)
        nc.vector.tensor_copy(sb[:], ps[:])
        nc.sync.dma_start(
            bass.AP(self.dr["scores"], n * 512, [[4, 128], [1, 4]]), sb[:]
        )

    # ---- full pipelined emission ---------------------------------------
    def emit_recurrence(self):
        T, NBLK = self.T, self.NBLK
        emitted = [False] * (T * 16 // 128)   # score chunks
        nsc = len(emitted)

        def ready_chunks(s):
            out = []
            for n in range(nsc):
                if not emitted[n] and max(8 * n + 7, (T - 1) - 8 * n) <= s:
                    out.append(n)
            return out

        def interleave(fa, fb):
            out = []
            for x, y in zip(fa, fb):
                out.append(x)
                out.append(y)
            return out

        def pair(blk):
            fi, fg, fm = self.production_items("f", blk)
            bi, bg, bm = self.production_items("b", NBLK - 1 - blk)
            gt = [fi, bi]
            for p in range(8):
                gt += fg[p] + bg[p]
            mm = [fm[p] + bm[p] for p in range(8)]  # per cons-pos, 6 items
            return gt, mm

        pairs = [pair(b) for b in range(NBLK)]
        # pre-emit: idx + gather/transpose + Wx-matmuls for the first two
        # consumption slabs of pair 0 (both dirs) so the chains can start.
        gt0, mm0 = pairs[0]
        for it in gt0[:14]:       # idx x2 + 2 slabs x 2 dirs x 3
            it()
        for it in mm0[0] + mm0[1]:
            it()
        for blk in range(NBLK):
            gt_todo = []
            n_front = 0
            if blk == 0:
                gt_todo += gt0[14:]
                n_front = len(gt_todo)
            if blk + 1 < NBLK:
                gt_todo += pairs[blk + 1][0]
            # current pair's Wx matmuls for cons-pos 2..7, paced so pos p is
            # fully emitted by slot 8*(p-2)+8 (2-slab lead, gb bufs=3)
            mmq = [x for p in range(2, 8) for x in pairs[blk][1][p]]
            mmq_next = []
            if blk + 1 < NBLK:
                mmq_next = pairs[blk + 1][1][0] + pairs[blk + 1][1][1]
            k = km = kn = 0
            for tin in range(64):
                s = blk * 64 + tin
                self.slot("f", s)
                dp = 2 * tin + 1
                want = max((dp * len(gt_todo)) // 128, min(n_front, 3 * dp))
                while k < want:
                    gt_todo[k]()
                    k += 1
                self.slot("b", (T - 1) - s)
                dp = 2 * tin + 2
                want = max((dp * len(gt_todo)) // 128, min(n_front, 3 * dp))
                while k < want:
                    gt_todo[k]()
                    k += 1
                want_mm = min(len(mmq), ((tin + 1) * 36) // 48)
                while km < want_mm:
                    mmq[km]()
                    km += 1
                if tin >= 48:
                    want_n = ((tin - 47) * len(mmq_next)) // 16
                    while kn < want_n:
                        mmq_next[kn]()
                        kn += 1
                for n in ready_chunks(s)[:2]:
                    self.score_chunk(n)
                    emitted[n] = True
            while k < len(gt_todo):
                gt_todo[k]()
                k += 1
            while km < len(mmq):
                mmq[km]()
                km += 1
            while kn < len(mmq_next):
                mmq_next[kn]()
                kn += 1
        for n in range(nsc):
            if not emitted[n]:
                self.score_chunk(n)
                emitted[n] = True


def host_lstm_shared(inp):
    """Batch-independent host arrays (weights etc)."""
    shared = {}
    perm = np.concatenate([np.arange(128), 128 + np.arange(128),
                           384 + np.arange(128), 256 + np.arange(128)])
    emb = np.asarray(inp["emb"]).astype(np.float32)
    # synthetic row 8000: W_i_b @ e* = -30 for every i-gate (freezes bwd state
    # through the pad prefix: sigma(i)=0 -> c=h=0)
    W_i_b = np.asarray(inp["W_ih_b"]).astype(np.float64)[0:128]
    e_star, *_ = np.linalg.lstsq(W_i_b, np.full(128, -30.0), rcond=None)
    emb16 = np.concatenate([emb, e_star[None, :].astype(np.float32)], axis=0)
    shared["emb16"] = emb16.astype(np.float16)
    for d, sfx in (("f", "_f"), ("b", "_b")):
        wih = np.asarray(inp[f"W_ih{sfx}"]).astype(np.float32)[perm].copy()
        whh = np.asarray(inp[f"W_hh{sfx}"]).astype(np.float32)[perm].copy()
        bias = (np.asarray(inp[f"b_ih{sfx}"]) +
                np.asarray(inp[f"b_hh{sfx}"])).astype(np.float32)[perm].copy()
        # tanh gate: pre-double (tanh(g) = 2*sigmoid(2g) - 1)
        wih[384:] *= 2.0
        whh[384:] *= 2.0
        bias[384:] *= 2.0
        shared[f"wihT_{d}0"] = np.ascontiguousarray(wih.T[:128]).astype(np.float16)
        shared[f"wihT_{d}1"] = np.ascontiguousarray(wih.T[128:]).astype(np.float16)
        shared[f"whhT_{d}"] = np.ascontiguousarray(whh.T).astype(np.float16)
        shared[f"biasK4_{d}"] = np.ascontiguousarray(
            bias.reshape(4, 128)).astype(np.float16)
        wl = np.asarray(inp["W_lab"]).astype(np.float32)
        half = wl[:, :128] if d == "f" else wl[:, 128:]
        shared[f"wlabT_{d}"] = np.ascontiguousarray(half.T).astype(np.float16)
    shared["ident"] = np.eye(128, dtype=np.float16)
    # ones4[k, col] = 1 iff the j-group of col == k   (col = tin*64 + j*16 + b)
    j_of_col = (np.arange(512) // 16) % 4
    shared["ones4"] = (j_of_col[None, :] == np.arange(4)[:, None]).astype(np.float16)
    return shared


def host_tokens(pad_seq, lens, T=512):
    """Per-core token arrays: fwd natural; bwd with pad positions remapped to
    the synthetic frozen-state row (8000)."""
    tok_f = np.ascontiguousarray(pad_seq.T).reshape(-1).astype(np.int32)
    tb = pad_seq.T.copy().astype(np.int32)          # [T, NB]
    invalid = np.arange(T)[:, None] >= lens[None, :]
    tb[invalid] = 8000
    tok_b = np.ascontiguousarray(tb).reshape(-1)
    return tok_f, tok_b


def lstm_dram_specs(T=512):
    return [
        ("emb16", [8001, 256], F16),
        ("tokens_f", [T * 16], I32), ("tokens_b", [T * 16], I32),
        ("wihT_f0", [128, 512], F16), ("wihT_f1", [128, 512], F16),
        ("wihT_b0", [128, 512], F16), ("wihT_b1", [128, 512], F16),
        ("whhT_f", [128, 512], F16), ("whhT_b", [128, 512], F16),
        ("biasK4_f", [4, 128], F16), ("biasK4_b", [4, 128], F16),
        ("wlabT_f", [128, 4], F16), ("wlabT_b", [128, 4], F16),
        ("ident", [128, 128], F16), ("ones4", [4, 512], F16),
    ]


def make_pools(ctx_persist, ctx_trans, tc):
    p = {}
    p["hist"] = ctx_persist.enter_context(tc.tile_pool(name="hist", bufs=1))
    p["state"] = ctx_persist.enter_context(tc.tile_pool(name="state", bufs=1))
    p["wts"] = ctx_persist.enter_context(tc.tile_pool(name="wts", bufs=1))
    p["idx"] = ctx_trans.enter_context(tc.tile_pool(name="idx", bufs=4))
    p["xg"] = ctx_trans.enter_context(tc.tile_pool(name="xg", bufs=6))
    p["xt"] = ctx_trans.enter_context(tc.tile_pool(name="xt", bufs=20))
    p["tp_ps"] = ctx_trans.enter_context(tc.tile_pool(name="tp_ps", bufs=1, space="PSUM"))
    p["gb_f"] = ctx_trans.enter_context(tc.tile_pool(name="gb_f", bufs=3, space="PSUM"))
    p["gb_b"] = ctx_trans.enter_context(tc.tile_pool(name="gb_b", bufs=3, space="PSUM"))
    p["sc_ps"] = ctx_trans.enter_context(tc.tile_pool(name="sc_ps", bufs=1, space="PSUM"))
    p["sc_sb"] = ctx_trans.enter_context(tc.tile_pool(name="sc_sb", bufs=4))
    for d in ("f", "b"):
        for nm in ("act", "m2", "t", "tc"):
            p[f"{nm}_{d}"] = ctx_trans.enter_context(
                tc.tile_pool(name=f"{nm}_{d}", bufs=2))
    return p


# ---------------------------------------------------------------------------
# DRAM declarations + host prep + SPMD driver
# ---------------------------------------------------------------------------

def _build_program():
    nc = bass.Bass(trn_type="TRN2")
    dr = {}
    for name, shape, dt in lstm_dram_specs(T):
        dr[name] = nc.dram_tensor(name, shape, dt, kind="ExternalInput")
    for name, shape, dt in CRF_DRAM_SPECS:
        dr[name] = nc.dram_tensor(name, shape, dt, kind="ExternalInput")
    for name, shape, dt in CRF_SCRATCH_SPECS:
        dr[name] = nc.dram_tensor(name, shape, dt)
    dr["scores"] = nc.dram_tensor("scores", [T * 16, 4], F32)
    dr["labels"] = nc.dram_tensor("labels", [NB, T], I32, kind="ExternalOutput")

    with tile.TileContext(nc) as tc:
        with ExitStack() as ctx:
            with ExitStack() as ctx_trans:
                pools = make_pools(ctx, ctx_trans, tc)
                em = LstmEmitter(nc, tc, dr, T, pools)
                em.emit_recurrence()
            with ExitStack() as ctx_crf:
                crf_pool = ctx_crf.enter_context(tc.tile_pool(name="crf", bufs=1))
                emit_crf(nc, tc, dr, crf_pool)
    return nc


_CACHE = {}
LAST_EXEC_NS = None


def kernel(**inputs):
    global LAST_EXEC_NS
    _apply_patches()
    from concourse.bass_utils import run_bass_kernel_spmd

    inp = {k: np.asarray(v) for k, v in inputs.items()}
    if "nc" not in _CACHE:
        _CACHE["nc"] = _build_program()
    nc = _CACHE["nc"]

    shared = host_lstm_shared(inp)

    trans = inp["transitions"].astype(np.float32)
    fromB = inp["from_BOS"].astype(np.float32)
    toEOS = inp["to_EOS"].astype(np.float32)
    b_lab = inp["b_lab"].astype(np.float32)

    pad_seq = inp["pad_seq"].astype(np.int64)
    lens_full = inp["lens"].astype(np.int64)

    in_maps = []
    for core in range(NCORES):
        b0 = core * NB
        seq = pad_seq[b0:b0 + NB]
        lens = lens_full[b0:b0 + NB]
        m = dict(shared)
        m["tokens_f"], m["tokens_b"] = host_tokens(seq, lens, T)
        m.update(host_crf_consts(lens, trans, fromB, toEOS, b_lab))
        in_maps.append(m)

    res = run_bass_kernel_spmd(nc, in_maps, list(range(NCORES)))
    LAST_EXEC_NS = res.exec_time_ns
    out = np.concatenate([res.results[c]["labels"] for c in range(NCORES)], axis=0)
    return out.astype(np.int32)
